# revision 6
# baseline (speedup 1.0000x reference)
"""Trainium2 Bass kernel for nn_Attention_9122510537215 (gnn_message_passing).

Math (per batch b):
    Q = query @ Wq.T + bq                  [LQ=256, 256]
    K = input @ Wk.T + bk                  [LK, 256]
    V = input @ Wv.T + bv                  [LK, 256]
    alpha = softmax_k(Q @ K.T / 16)        [256, LK]
    out[j] = sum_k alpha[j, k] * V[k, j]   [256]

Restructure vs the two-layout baseline:
  * bk shifts every score column by a constant along k -> cancels in softmax_k.
  * G[b] = Wk.T @ (query_b @ Wq.T + bq).T / 16, so s[q, k] = (G.T @ x.T)[q, k].
  * vT[j, k] = (Wv @ x.T)[j, k] is computed ON DEVICE from the SAME moving
    operand as the scores (x.T), with Wv.T stationary.  Then
        numer[j] = sum_k e[j, k] * vT[j, k],   denom[j] = sum_k e[j, k]
    and out = numer / denom + bv (bv applied on host; scores are O(1) so the
    softmax runs unnormalized without max-subtraction).
  * Only ONE layout of the input is shipped (x.T, features-on-partitions):
    half the HBM traffic of the baseline.  G and Wv.T are the only PE
    stationaries (8 loads per 512-column chunk, fully hidden), the moving
    stream is x.T.
  * e = exp(s) runs on ScalarE with a fused free-axis accumulate (denom for
    free).  numer's multiply+reduce is split between VectorE
    (tensor_tensor_reduce) and GpSimd (scalar_tensor_tensor) so no single
    engine bottlenecks.  G / Wv are pre-scaled by 16 on the host (exp applies
    scale=1/16; numer is divided by 16 on the host).

Distribution: the LK (node) axis is zero-padded to 50176 = 8 * 6272 and
sharded across the 8 NeuronCores; each core returns per-chunk column sums
[B, 128, 2(half), 2(numer/denom), NCOL] fp32 and the host reduces in float64.
Padded rows have x = 0 -> s = 0 -> e = 1 exactly, contributing 0 to numer and
+176 (total, last core only) to denom: subtracted exactly on the host.
"""

import numpy as np
from contextlib import ExitStack

import ml_dtypes

import concourse.bass as bass
import concourse.mybir as mybir
import concourse.tile as tile
from concourse import bacc
from concourse.bass_utils import run_bass_kernel_spmd

# Problem constants (hardcoded; kernel.py must be self-contained).
B = 4
LQ = 256
LK = 50000
OUT = 256
KV = 256            # input feature dim
NORM = 1.0 / 16.0   # 1/sqrt(OUT)
PRESCALE = 16.0     # host multiplies G and Wv by this; undone on device/host

N_CORES = 8
KS = 6272                  # nodes per core per batch (49 * 128)
LK_PAD = KS * N_CORES      # 50176
N_PAD = LK_PAD - LK        # 176 zero rows, all on the last core
CHUNK = 512                # moving columns per PSUM bank

F16 = mybir.dt.float16
F32 = mybir.dt.float32
F8 = mybir.dt.float8e4

ALU = mybir.AluOpType
AF = mybir.ActivationFunctionType


def _chunks(ks):
    out = []
    c0 = 0
    while c0 < ks:
        out.append((c0, min(CHUNK, ks - c0)))
        c0 += CHUNK
    return out


def build(ks=KS, fp8=False):
    """Emit the per-core SPMD Bass module (identical on all cores).

    fp8: x / G / Wv are fp8e4 and the four matmuls per chunk run in DoubleRow
    mode (contraction 256 in one pass).  Otherwise fp16.
    """
    chunks = _chunks(ks)
    ncol = len(chunks)
    DT = F8 if fp8 else F16

    nc = bacc.Bacc("TRN2", target_bir_lowering=False, debug=False,
                   num_devices=N_CORES)
    if fp8:
        # DoubleRow operand layouts: [partition p, slot o, cols]; contraction
        # index i = o * 128 + p.
        xt = nc.dram_tensor("xt", [B, 128, 2, ks], DT, kind="ExternalInput")
        g = nc.dram_tensor("g", [B, 128, 2, 256], DT, kind="ExternalInput")
        wv = nc.dram_tensor("wv", [128, 2, 256], DT, kind="ExternalInput")
    else:
        # [b, i-half, i-partition, cols]
        xt = nc.dram_tensor("xt", [B, 2, 128, ks], DT, kind="ExternalInput")
        g = nc.dram_tensor("g", [B, 2, 128, 256], DT, kind="ExternalInput")
        wv = nc.dram_tensor("wv", [2, 128, 256], DT, kind="ExternalInput")
    oc = nc.dram_tensor("oc", [B, 128, 2, 2, ncol], F32, kind="ExternalOutput")

    with ExitStack() as ctx:
        tc = ctx.enter_context(tile.TileContext(nc))
        wp = ctx.enter_context(tc.tile_pool(name="wp", bufs=1))
        xp = ctx.enter_context(tc.tile_pool(name="xp", bufs=1))
        pp = ctx.enter_context(tc.tile_pool(name="pp", bufs=2, space="PSUM"))
        ep = ctx.enter_context(tc.tile_pool(name="ep", bufs=3))
        sp = ctx.enter_context(tc.tile_pool(name="sp", bufs=2))
        ocp = ctx.enter_context(tc.tile_pool(name="ocp", bufs=2))

        if fp8:
            g_sb = wp.tile([128, B, 2, 256], DT, tag="g")
            wv_sb = wp.tile([128, 2, 256], DT, tag="wv")
            x_sb = xp.tile([128, B, 2, ks], DT, tag="x")
            for b in range(B):
                nc.scalar.dma_start(out=g_sb[:, b], in_=g[b])
            nc.scalar.dma_start(out=wv_sb[:, :, :], in_=wv[:, :, :])
            for b in range(B):
                nc.sync.dma_start(out=x_sb[:, b], in_=xt[b])

            def mm_s(out_ap, b, qh, c0, csz):
                nc.tensor.matmul(
                    out_ap,
                    g_sb[:, b, :, qh * 128:(qh + 1) * 128],
                    x_sb[:, b, :, c0:c0 + csz],
                    start=True, stop=True,
                    perf_mode=mybir.MatmulPerfMode.DoubleRow)

            def mm_v(out_ap, b, jh, c0, csz):
                nc.tensor.matmul(
                    out_ap,
                    wv_sb[:, :, jh * 128:(jh + 1) * 128],
                    x_sb[:, b, :, c0:c0 + csz],
                    start=True, stop=True,
                    perf_mode=mybir.MatmulPerfMode.DoubleRow)
        else:
            g_sb = wp.tile([128, B, 2, 256], DT, tag="g")
            wv_sb = wp.tile([128, 2, 256], DT, tag="wv")
            x_sb = xp.tile([128, B, 2, ks], DT, tag="x")
            for b in range(B):
                for ih in range(2):
                    nc.scalar.dma_start(out=g_sb[:, b, ih], in_=g[b, ih])
            for ih in range(2):
                nc.scalar.dma_start(out=wv_sb[:, ih], in_=wv[ih])
            for b in range(B):
                for ih in range(2):
                    nc.sync.dma_start(out=x_sb[:, b, ih], in_=xt[b, ih])

            def mm_s(out_ap, b, qh, c0, csz):
                for ih in range(2):
                    nc.tensor.matmul(
                        out_ap,
                        g_sb[:, b, ih, qh * 128:(qh + 1) * 128],
                        x_sb[:, b, ih, c0:c0 + csz],
                        start=(ih == 0), stop=(ih == 1))

            def mm_v(out_ap, b, jh, c0, csz):
                for ih in range(2):
                    nc.tensor.matmul(
                        out_ap,
                        wv_sb[:, ih, jh * 128:(jh + 1) * 128],
                        x_sb[:, b, ih, c0:c0 + csz],
                        start=(ih == 0), stop=(ih == 1))

        for b in range(B):
            occ = ocp.tile([128, 2, 2, ncol], F32, tag="occ")
            for t, (c0, csz) in enumerate(chunks):
                s_t = [pp.tile([128, CHUNK], F32, tag=f"s{h}", name=f"s{h}")
                       for h in range(2)]
                v_t = [pp.tile([128, CHUNK], F32, tag=f"v{h}", name=f"v{h}")
                       for h in range(2)]
                for h in range(2):
                    mm_s(s_t[h][:, :csz], b, h, c0, csz)
                    mm_v(v_t[h][:, :csz], b, h, c0, csz)
                e_t = []
                for h in range(2):
                    e_ = ep.tile([128, CHUNK], F16, tag=f"e{h}")
                    nc.scalar.activation(
                        e_[:, :csz], s_t[h][:, :csz], AF.Exp,
                        scale=1.0 / PRESCALE,
                        accum_out=occ[:, h, 1, t:t + 1])
                    e_t.append(e_)
                # numer: fused multiply+sum on VectorE via the standard
                # TensorScalarPtr instruction (GpSimd cannot read PSUM on real
                # HW, and the custom-ucode tensor_tensor_reduce faults there)
                for h in range(2):
                    p_ = sp.tile([128, CHUNK], F16, tag=f"p{h}", name=f"p{h}")
                    nc.vector.scalar_tensor_tensor(
                        out=p_[:, :csz],
                        in0=v_t[h][:, :csz], scalar=1.0, in1=e_t[h][:, :csz],
                        op0=ALU.mult, op1=ALU.mult,
                        accum_out=occ[:, h, 0, t:t + 1])
            nc.sync.dma_start(out=oc[b], in_=occ[:, :, :, :])
    nc.compile()
    return nc


def _to_fp8(a):
    return np.clip(a, -240.0, 240.0).astype(ml_dtypes.float8_e4m3)


def _prepare_inputs(query, input, Wq, bq, Wk, Wv, fp8=False):
    """Host-side marshalling: G (incl. bq, 1/16, PRESCALE), Wv.T, x.T shards."""
    Q = query.astype(np.float64) @ Wq.T.astype(np.float64) + bq
    G = np.einsum('di,bqd->biq', Wk.astype(np.float64), Q) * (NORM * PRESCALE)
    WvT = Wv.T.astype(np.float64) * PRESCALE           # [i, j]

    xpad = np.zeros((B, LK_PAD, KV), np.float32)
    xpad[:, :LK] = input
    xT = xpad.transpose(0, 2, 1)                       # [B, 256, LK_PAD] view

    if fp8:
        g8 = _to_fp8(G.reshape(B, 2, 128, 256).transpose(0, 2, 1, 3))
        wv8 = _to_fp8(WvT.reshape(2, 128, 256).transpose(1, 0, 2))
        g8 = np.ascontiguousarray(g8)
        wv8 = np.ascontiguousarray(wv8)
        in_maps = []
        for c in range(N_CORES):
            sl = slice(c * KS, (c + 1) * KS)
            xc = xT[:, :, sl].reshape(B, 2, 128, KS).transpose(0, 2, 1, 3)
            in_maps.append({"xt": np.ascontiguousarray(_to_fp8(xc)),
                            "g": g8, "wv": wv8})
    else:
        g16 = np.ascontiguousarray(
            G.astype(np.float32).astype(np.float16).reshape(B, 2, 128, 256))
        wv16 = np.ascontiguousarray(
            WvT.astype(np.float32).astype(np.float16).reshape(2, 128, 256))
        in_maps = []
        for c in range(N_CORES):
            sl = slice(c * KS, (c + 1) * KS)
            xc = xT[:, :, sl].reshape(B, 2, 128, KS)
            in_maps.append({"xt": np.ascontiguousarray(xc.astype(np.float16)),
                            "g": g16, "wv": wv16})
    return in_maps


USE_FP8 = True


def kernel(query, input, Wq, bq, Wk, bk, Wv, bv):
    # bk provably cancels in softmax over k; bq is folded into G; bv is applied
    # in the host-side epilogue below.
    query = np.asarray(query, dtype=np.float32)
    input = np.asarray(input, dtype=np.float32)
    Wq = np.asarray(Wq, dtype=np.float32)
    bq = np.asarray(bq, dtype=np.float32)
    Wk = np.asarray(Wk, dtype=np.float32)
    Wv = np.asarray(Wv, dtype=np.float32)
    bv = np.asarray(bv, dtype=np.float32)

    nc = build(fp8=USE_FP8)
    in_maps = _prepare_inputs(query, input, Wq, bq, Wk, Wv, fp8=USE_FP8)
    res = run_bass_kernel_spmd(nc, in_maps, core_ids=list(range(N_CORES)))
    kernel._last_result = res

    numer = np.zeros((B, 2, 128))
    denom = np.zeros((B, 2, 128))
    for r in res.results:
        o = r["oc"].astype(np.float64)       # [B, 128, 2, 2, ncol]
        numer += o[:, :, :, 0, :].sum(axis=3).transpose(0, 2, 1)
        denom += o[:, :, :, 1, :].sum(axis=3).transpose(0, 2, 1)
    numer = numer.reshape(B, OUT) / PRESCALE
    denom = denom.reshape(B, OUT) - N_PAD    # padded rows contribute e=1 each
    out = numer / denom + bv
    return out.astype(np.float32)


if __name__ == "__main__":
    # CoreSim smoke test on a reduced size (2.25 chunks -> [512, 512, 128]).
    from concourse.bass_interp import CoreSim

    for fp8 in (False, True):
        ks = 1152
        rng = np.random.default_rng(0)
        x = rng.standard_normal((B, ks, KV)).astype(np.float32)
        G = (rng.standard_normal((B, KV, 256)) * 0.4).astype(np.float64)
        WvT = (rng.standard_normal((KV, 256)) * 0.8).astype(np.float64)

        nc = build(ks=ks, fp8=fp8)
        sim = CoreSim(nc)
        xT = x.transpose(0, 2, 1)  # [B, 256, ks]
        if fp8:
            sim.tensor("xt")[:] = _to_fp8(
                xT.reshape(B, 2, 128, ks).transpose(0, 2, 1, 3))
            sim.tensor("g")[:] = _to_fp8(
                G.reshape(B, 2, 128, 256).transpose(0, 2, 1, 3))
            sim.tensor("wv")[:] = _to_fp8(
                WvT.reshape(2, 128, 256).transpose(1, 0, 2))
            xq = _to_fp8(xT).astype(np.float64)
            gq = _to_fp8(G).astype(np.float64)
            wq = _to_fp8(WvT).astype(np.float64)
        else:
            sim.tensor("xt")[:] = xT.reshape(B, 2, 128, ks).astype(np.float16)
            sim.tensor("g")[:] = G.astype(np.float16).reshape(B, 2, 128, 256)
            sim.tensor("wv")[:] = WvT.astype(np.float16).reshape(2, 128, 256)
            xq = xT.astype(np.float16).astype(np.float64)
            gq = G.astype(np.float16).astype(np.float64)
            wq = WvT.astype(np.float16).astype(np.float64)
        sim.simulate()
        got = np.array(sim.tensor("oc")).astype(np.float64)  # [B,128,2,2,ncol]
        gnum = got[:, :, :, 0, :].sum(axis=3).transpose(0, 2, 1).reshape(B, 256)
        gden = got[:, :, :, 1, :].sum(axis=3).transpose(0, 2, 1).reshape(B, 256)

        wnum = np.zeros((B, 256))
        wden = np.zeros((B, 256))
        for b in range(B):
            s = (gq[b].T @ xq[b]) / PRESCALE          # [256 q, ks]
            e = np.exp(s)
            v = wq.T @ xq[b]                          # [256 j, ks]
            e16 = e.astype(np.float16).astype(np.float64)
            wnum[b] = (e16 * v).sum(axis=1)
            wden[b] = e16.sum(axis=1)
        en = np.abs(gnum - wnum).max() / np.abs(wnum).max()
        ed = np.abs(gden - wden).max() / np.abs(wden).max()
        print(f"fp8={fp8}: CoreSim numer rel err {en:.3e}, denom rel err {ed:.3e}")
        assert en < 2e-2 and ed < 2e-2, (en, ed)
    print("OK")


# revision 14
# speedup vs baseline: 1.0109x; 1.0109x over previous
"""Trainium2 Bass kernel for nn_Attention_9122510537215 (gnn_message_passing).

Math (per batch b):
    Q = query @ Wq.T + bq                  [LQ=256, 256]
    K = input @ Wk.T + bk                  [LK, 256]
    V = input @ Wv.T + bv                  [LK, 256]
    alpha = softmax_k(Q @ K.T / 16)        [256, LK]
    out[j] = sum_k alpha[j, k] * V[k, j]   [256]

Restructure vs the two-layout baseline:
  * bk shifts every score column by a constant along k -> cancels in softmax_k.
  * G[b] = Wk.T @ (query_b @ Wq.T + bq).T / 16, so s[q, k] = (G.T @ x.T)[q, k].
  * vT[j, k] = (Wv @ x.T)[j, k] is computed ON DEVICE from the SAME moving
    operand as the scores (x.T), with Wv.T stationary.  Then
        numer[j] = sum_k e[j, k] * vT[j, k],   denom[j] = sum_k e[j, k]
    and out = numer / denom + bv (bv applied on host; scores are O(1) so the
    softmax runs unnormalized without max-subtraction).
  * Only ONE layout of the input is shipped (x.T, features-on-partitions):
    half the HBM traffic of the baseline.  G and Wv.T are the only PE
    stationaries (8 loads per 512-column chunk, fully hidden), the moving
    stream is x.T.
  * e = exp(s) runs on ScalarE with a fused free-axis accumulate (denom for
    free).  numer's multiply+reduce is split between VectorE
    (tensor_tensor_reduce) and GpSimd (scalar_tensor_tensor) so no single
    engine bottlenecks.  G / Wv are pre-scaled by 16 on the host (exp applies
    scale=1/16; numer is divided by 16 on the host).

Distribution: the LK (node) axis is zero-padded to 50176 = 8 * 6272 and
sharded across the 8 NeuronCores; each core returns per-chunk column sums
[B, 128, 2(half), 2(numer/denom), NCOL] fp32 and the host reduces in float64.
Padded rows have x = 0 -> s = 0 -> e = 1 exactly, contributing 0 to numer and
+176 (total, last core only) to denom: subtracted exactly on the host.
"""

import numpy as np
from contextlib import ExitStack

import ml_dtypes

import concourse.bass as bass
import concourse.mybir as mybir
import concourse.tile as tile
from concourse import bacc
from concourse.bass_utils import run_bass_kernel_spmd

# Problem constants (hardcoded; kernel.py must be self-contained).
B = 4
LQ = 256
LK = 50000
OUT = 256
KV = 256            # input feature dim
NORM = 1.0 / 16.0   # 1/sqrt(OUT)
PRESCALE = 128.0    # host multiplies G and Wv by this; undone on device/host
                    # (keeps the fp8 hi+residual split in e4m3's normal range)

N_CORES = 8
KS = 6272                  # nodes per core per batch (49 * 128)
LK_PAD = KS * N_CORES      # 50176
N_PAD = LK_PAD - LK        # 176 zero rows, all on the last core
CHUNK = 512                # moving columns per PSUM bank

F16 = mybir.dt.float16
F32 = mybir.dt.float32
F8 = mybir.dt.float8e4

ALU = mybir.AluOpType
AF = mybir.ActivationFunctionType


def _pairs(ks):
    """k-range split into pairs of CHUNK-wide subchunks (one PSUM bank each)."""
    out = []
    c0 = 0
    while c0 < ks:
        sub = []
        for _ in range(2):
            if c0 < ks:
                sub.append((c0, min(CHUNK, ks - c0)))
                c0 += CHUNK
        out.append(sub)
    return out


def build(ks=KS, fp8=None):
    """Emit the per-core SPMD Bass module (identical on all cores).

    fp8: x / G / Wv are fp8e4 and the four matmuls per chunk run in DoubleRow
    mode (contraction 256 in one pass).  Otherwise fp16.
    """
    if fp8 is None:
        fp8 = USE_FP8
    chunks = _chunks(ks)
    ncol = len(chunks)
    DT = F8 if fp8 else F16

    nc = bacc.Bacc("TRN2", target_bir_lowering=False, debug=False,
                   num_devices=N_CORES)
    if fp8:
        # DoubleRow operand layouts: [partition p, slot o, cols]; contraction
        # index i = o * 128 + p.  g/wv carry the fp8 "hi" part; gr/wvr the
        # fp8 residual (G_pre - hi), accumulated in a second DoubleRow pass.
        xt = nc.dram_tensor("xt", [B, 128, 2, ks], DT, kind="ExternalInput")
        g = nc.dram_tensor("g", [B, 128, 2, 256], DT, kind="ExternalInput")
        gr = nc.dram_tensor("gr", [B, 128, 2, 256], DT, kind="ExternalInput")
        wv = nc.dram_tensor("wv", [128, 2, 256], DT, kind="ExternalInput")
        wvr = nc.dram_tensor("wvr", [128, 2, 256], DT, kind="ExternalInput")
    else:
        # [b, i-half, i-partition, cols]
        xt = nc.dram_tensor("xt", [B, 2, 128, ks], DT, kind="ExternalInput")
        g = nc.dram_tensor("g", [B, 2, 128, 256], DT, kind="ExternalInput")
        wv = nc.dram_tensor("wv", [2, 128, 256], DT, kind="ExternalInput")
    oc = nc.dram_tensor("oc", [B, 128, 2, 2, ncol], F32, kind="ExternalOutput")

    with ExitStack() as ctx:
        tc = ctx.enter_context(tile.TileContext(nc))
        wp = ctx.enter_context(tc.tile_pool(name="wp", bufs=1))
        xp = ctx.enter_context(tc.tile_pool(name="xp", bufs=1))
        pp = ctx.enter_context(tc.tile_pool(name="pp", bufs=2, space="PSUM"))
        ep = ctx.enter_context(tc.tile_pool(name="ep", bufs=3))
        sp = ctx.enter_context(tc.tile_pool(name="sp", bufs=2))
        ocp = ctx.enter_context(tc.tile_pool(name="ocp", bufs=2))

        if fp8:
            g_sb = wp.tile([128, 2, B, 2, 256], DT, tag="g")
            wv_sb = wp.tile([128, 2, 2, 256], DT, tag="wv")
            x_sb = xp.tile([128, B, 2, ks], DT, tag="x")
            for b in range(B):
                nc.scalar.dma_start(out=g_sb[:, 0, b], in_=g[b])
                nc.scalar.dma_start(out=g_sb[:, 1, b], in_=gr[b])
            nc.scalar.dma_start(out=wv_sb[:, 0], in_=wv[:, :, :])
            nc.scalar.dma_start(out=wv_sb[:, 1], in_=wvr[:, :, :])
            for b in range(B):
                nc.sync.dma_start(out=x_sb[:, b], in_=xt[b])

            def mm_s(out_ap, b, qh, c0, csz):
                for r in range(2):
                    nc.tensor.matmul(
                        out_ap,
                        g_sb[:, r, b, :, qh * 128:(qh + 1) * 128],
                        x_sb[:, b, :, c0:c0 + csz],
                        start=(r == 0), stop=(r == 1),
                        perf_mode=mybir.MatmulPerfMode.DoubleRow)

            def mm_v(out_ap, b, jh, c0, csz):
                for r in range(2):
                    nc.tensor.matmul(
                        out_ap,
                        wv_sb[:, r, :, jh * 128:(jh + 1) * 128],
                        x_sb[:, b, :, c0:c0 + csz],
                        start=(r == 0), stop=(r == 1),
                        perf_mode=mybir.MatmulPerfMode.DoubleRow)
        else:
            g_sb = wp.tile([128, B, 2, 256], DT, tag="g")
            wv_sb = wp.tile([128, 2, 256], DT, tag="wv")
            x_sb = xp.tile([128, B, 2, ks], DT, tag="x")
            for b in range(B):
                for ih in range(2):
                    nc.scalar.dma_start(out=g_sb[:, b, ih], in_=g[b, ih])
            for ih in range(2):
                nc.scalar.dma_start(out=wv_sb[:, ih], in_=wv[ih])
            for b in range(B):
                for ih in range(2):
                    nc.sync.dma_start(out=x_sb[:, b, ih], in_=xt[b, ih])

            def mm_s(out_ap, b, qh, c0, csz):
                for ih in range(2):
                    nc.tensor.matmul(
                        out_ap,
                        g_sb[:, b, ih, qh * 128:(qh + 1) * 128],
                        x_sb[:, b, ih, c0:c0 + csz],
                        start=(ih == 0), stop=(ih == 1))

            def mm_v(out_ap, b, jh, c0, csz):
                for ih in range(2):
                    nc.tensor.matmul(
                        out_ap,
                        wv_sb[:, ih, jh * 128:(jh + 1) * 128],
                        x_sb[:, b, ih, c0:c0 + csz],
                        start=(ih == 0), stop=(ih == 1))

        for b in range(B):
            occ = ocp.tile([128, 2, 2, ncol], F32, tag="occ")
            for t, (c0, csz) in enumerate(chunks):
                s_t = [pp.tile([128, CHUNK], F32, tag=f"s{h}", name=f"s{h}")
                       for h in range(2)]
                v_t = [pp.tile([128, CHUNK], F32, tag=f"v{h}", name=f"v{h}")
                       for h in range(2)]
                for h in range(2):
                    mm_s(s_t[h][:, :csz], b, h, c0, csz)
                    mm_v(v_t[h][:, :csz], b, h, c0, csz)
                e_t = []
                for h in range(2):
                    e_ = ep.tile([128, CHUNK], F16, tag=f"e{h}")
                    nc.scalar.activation(
                        e_[:, :csz], s_t[h][:, :csz], AF.Exp,
                        scale=1.0 / PRESCALE,
                        accum_out=occ[:, h, 1, t:t + 1])
                    e_t.append(e_)
                # numer: fused multiply+sum on VectorE via the standard
                # TensorScalarPtr instruction (GpSimd cannot read PSUM on real
                # HW, and the custom-ucode tensor_tensor_reduce faults there)
                for h in range(2):
                    p_ = sp.tile([128, CHUNK], F16, tag=f"p{h}", name=f"p{h}")
                    nc.vector.scalar_tensor_tensor(
                        out=p_[:, :csz],
                        in0=v_t[h][:, :csz], scalar=1.0, in1=e_t[h][:, :csz],
                        op0=ALU.mult, op1=ALU.mult,
                        accum_out=occ[:, h, 0, t:t + 1])
            nc.sync.dma_start(out=oc[b], in_=occ[:, :, :, :])
    nc.compile()
    return nc


def _to_fp8(a):
    return np.clip(a, -240.0, 240.0).astype(ml_dtypes.float8_e4m3)


def _prepare_inputs(query, input, Wq, bq, Wk, Wv, fp8=False):
    """Host-side marshalling: G (incl. bq, 1/16, PRESCALE), Wv.T, x.T shards."""
    Q = query.astype(np.float64) @ Wq.T.astype(np.float64) + bq
    G = np.einsum('di,bqd->biq', Wk.astype(np.float64), Q) * (NORM * PRESCALE)
    WvT = Wv.T.astype(np.float64) * PRESCALE           # [i, j]

    xpad = np.zeros((B, LK_PAD, KV), np.float32)
    xpad[:, :LK] = input
    xT = xpad.transpose(0, 2, 1)                       # [B, 256, LK_PAD] view

    if fp8:
        def hires(a):  # [.., 2slots, ..] DoubleRow layout + residual split
            hi = _to_fp8(a)
            res = _to_fp8(a - hi.astype(np.float64))
            return np.ascontiguousarray(hi), np.ascontiguousarray(res)

        g8, gr8 = hires(G.reshape(B, 2, 128, 256).transpose(0, 2, 1, 3))
        wv8, wvr8 = hires(WvT.reshape(2, 128, 256).transpose(1, 0, 2))
        in_maps = []
        for c in range(N_CORES):
            sl = slice(c * KS, (c + 1) * KS)
            xc = xT[:, :, sl].reshape(B, 2, 128, KS).transpose(0, 2, 1, 3)
            in_maps.append({"xt": np.ascontiguousarray(_to_fp8(xc)),
                            "g": g8, "gr": gr8, "wv": wv8, "wvr": wvr8})
    else:
        g16 = np.ascontiguousarray(
            G.astype(np.float32).astype(np.float16).reshape(B, 2, 128, 256))
        wv16 = np.ascontiguousarray(
            WvT.astype(np.float32).astype(np.float16).reshape(2, 128, 256))
        in_maps = []
        for c in range(N_CORES):
            sl = slice(c * KS, (c + 1) * KS)
            xc = xT[:, :, sl].reshape(B, 2, 128, KS)
            in_maps.append({"xt": np.ascontiguousarray(xc.astype(np.float16)),
                            "g": g16, "wv": wv16})
    return in_maps


USE_FP8 = True


def kernel(query, input, Wq, bq, Wk, bk, Wv, bv):
    # bk provably cancels in softmax over k; bq is folded into G; bv is applied
    # in the host-side epilogue below.
    query = np.asarray(query, dtype=np.float32)
    input = np.asarray(input, dtype=np.float32)
    Wq = np.asarray(Wq, dtype=np.float32)
    bq = np.asarray(bq, dtype=np.float32)
    Wk = np.asarray(Wk, dtype=np.float32)
    Wv = np.asarray(Wv, dtype=np.float32)
    bv = np.asarray(bv, dtype=np.float32)

    nc = build(fp8=USE_FP8)
    in_maps = _prepare_inputs(query, input, Wq, bq, Wk, Wv, fp8=USE_FP8)
    res = run_bass_kernel_spmd(nc, in_maps, core_ids=list(range(N_CORES)))
    kernel._last_result = res

    numer = np.zeros((B, 2, 128))
    denom = np.zeros((B, 2, 128))
    for r in res.results:
        o = r["oc"].astype(np.float64)       # [B, 128, 2, 2, ncol]
        numer += o[:, :, :, 0, :].sum(axis=3).transpose(0, 2, 1)
        denom += o[:, :, :, 1, :].sum(axis=3).transpose(0, 2, 1)
    numer = numer.reshape(B, OUT) / PRESCALE
    denom = denom.reshape(B, OUT) - N_PAD    # padded rows contribute e=1 each
    out = numer / denom + bv
    return out.astype(np.float32)


if __name__ == "__main__":
    # CoreSim smoke test on a reduced size (2.25 chunks -> [512, 512, 128]).
    from concourse.bass_interp import CoreSim

    for fp8 in (False, True):
        ks = 1152
        rng = np.random.default_rng(0)
        x = rng.standard_normal((B, ks, KV)).astype(np.float32)
        G = (rng.standard_normal((B, KV, 256)) * 0.4).astype(np.float64)
        WvT = (rng.standard_normal((KV, 256)) * 0.8).astype(np.float64)

        nc = build(ks=ks, fp8=fp8)
        sim = CoreSim(nc)
        xT = x.transpose(0, 2, 1)  # [B, 256, ks]
        if fp8:
            sim.tensor("xt")[:] = _to_fp8(
                xT.reshape(B, 2, 128, ks).transpose(0, 2, 1, 3))
            gdr = G.reshape(B, 2, 128, 256).transpose(0, 2, 1, 3)
            wdr = WvT.reshape(2, 128, 256).transpose(1, 0, 2)
            g_hi = _to_fp8(gdr)
            g_re = _to_fp8(gdr - g_hi.astype(np.float64))
            w_hi = _to_fp8(wdr)
            w_re = _to_fp8(wdr - w_hi.astype(np.float64))
            sim.tensor("g")[:] = g_hi
            sim.tensor("gr")[:] = g_re
            sim.tensor("wv")[:] = w_hi
            sim.tensor("wvr")[:] = w_re
            xq = _to_fp8(xT).astype(np.float64)
            gq = (g_hi.astype(np.float64) + g_re.astype(np.float64)
                  ).transpose(0, 2, 1, 3).reshape(B, 256, 256)
            wq = (w_hi.astype(np.float64) + w_re.astype(np.float64)
                  ).transpose(1, 0, 2).reshape(256, 256)
        else:
            sim.tensor("xt")[:] = xT.reshape(B, 2, 128, ks).astype(np.float16)
            sim.tensor("g")[:] = G.astype(np.float16).reshape(B, 2, 128, 256)
            sim.tensor("wv")[:] = WvT.astype(np.float16).reshape(2, 128, 256)
            xq = xT.astype(np.float16).astype(np.float64)
            gq = G.astype(np.float16).astype(np.float64)
            wq = WvT.astype(np.float16).astype(np.float64)
        sim.simulate()
        got = np.array(sim.tensor("oc")).astype(np.float64)  # [B,128,2,2,ncol]
        gnum = got[:, :, :, 0, :].sum(axis=3).transpose(0, 2, 1).reshape(B, 256)
        gden = got[:, :, :, 1, :].sum(axis=3).transpose(0, 2, 1).reshape(B, 256)

        wnum = np.zeros((B, 256))
        wden = np.zeros((B, 256))
        for b in range(B):
            s = (gq[b].T @ xq[b]) / PRESCALE          # [256 q, ks]
            e = np.exp(s)
            v = wq.T @ xq[b]                          # [256 j, ks]
            e16 = e.astype(np.float16).astype(np.float64)
            wnum[b] = (e16 * v).sum(axis=1)
            wden[b] = e16.sum(axis=1)
        en = np.abs(gnum - wnum).max() / np.abs(wnum).max()
        ed = np.abs(gden - wden).max() / np.abs(wden).max()
        print(f"fp8={fp8}: CoreSim numer rel err {en:.3e}, denom rel err {ed:.3e}")
        assert en < 2e-2 and ed < 2e-2, (en, ed)
    print("OK")


# revision 24
# speedup vs baseline: 1.3632x; 1.3485x over previous
"""Trainium2 Bass kernel for nn_Attention_9122510537215 (gnn_message_passing).

Math (per batch b):
    Q = query @ Wq.T + bq                  [LQ=256, 256]
    K = input @ Wk.T + bk                  [LK, 256]
    V = input @ Wv.T + bv                  [LK, 256]
    alpha = softmax_k(Q @ K.T / 16)        [256, LK]
    out[j] = sum_k alpha[j, k] * V[k, j]   [256]

Restructure vs the two-layout baseline:
  * bk shifts every score column by a constant along k -> cancels in softmax_k.
  * G[b] = Wk.T @ (query_b @ Wq.T + bq).T / 16, so s[q, k] = (G.T @ x.T)[q, k].
  * vT[j, k] = (Wv @ x.T)[j, k] is computed ON DEVICE from the SAME moving
    operand as the scores (x.T), with Wv.T stationary.  Then
        numer[j] = sum_k e[j, k] * vT[j, k],   denom[j] = sum_k e[j, k]
    and out = numer / denom + bv (bv applied on host; scores are O(1) so the
    softmax runs unnormalized without max-subtraction).
  * Only ONE layout of the input is shipped (x.T, features-on-partitions):
    half the HBM traffic of the baseline.  G and Wv.T are the only PE
    stationaries (8 loads per 512-column chunk, fully hidden), the moving
    stream is x.T.
  * e = exp(s) runs on ScalarE with a fused free-axis accumulate (denom for
    free).  numer's multiply+reduce is split between VectorE
    (tensor_tensor_reduce) and GpSimd (scalar_tensor_tensor) so no single
    engine bottlenecks.  G / Wv are pre-scaled by 16 on the host (exp applies
    scale=1/16; numer is divided by 16 on the host).

Distribution: the LK (node) axis is zero-padded to 50176 = 8 * 6272 and
sharded across the 8 NeuronCores; each core returns per-chunk column sums
[B, 128, 2(half), 2(numer/denom), NCOL] fp32 and the host reduces in float64.
Padded rows have x = 0 -> s = 0 -> e = 1 exactly, contributing 0 to numer and
+176 (total, last core only) to denom: subtracted exactly on the host.
"""

import numpy as np
from contextlib import ExitStack

import ml_dtypes

import concourse.bass as bass
import concourse.mybir as mybir
import concourse.tile as tile
from concourse import bacc
from concourse.bass_utils import run_bass_kernel_spmd

# Problem constants (hardcoded; kernel.py must be self-contained).
B = 4
LQ = 256
LK = 50000
OUT = 256
KV = 256            # input feature dim
NORM = 1.0 / 16.0   # 1/sqrt(OUT)
PRESCALE = 128.0    # host multiplies G and Wv by this; undone on device/host
                    # (keeps the fp8 hi+residual split in e4m3's normal range)

N_CORES = 8
KS = 6272                  # nodes per core per batch (49 * 128)
LK_PAD = KS * N_CORES      # 50176
N_PAD = LK_PAD - LK        # 176 zero rows, all on the last core
CHUNK = 512                # moving columns per PSUM bank

F16 = mybir.dt.float16
F32 = mybir.dt.float32
F8 = mybir.dt.float8e4

ALU = mybir.AluOpType
AF = mybir.ActivationFunctionType


def _pairs(ks):
    """k-range split into pairs of CHUNK-wide subchunks (one PSUM bank each)."""
    out = []
    c0 = 0
    while c0 < ks:
        sub = []
        for _ in range(2):
            if c0 < ks:
                sub.append((c0, min(CHUNK, ks - c0)))
                c0 += CHUNK
        out.append(sub)
    return out


def build(ks=KS, fp8=None):
    """Emit the per-core SPMD Bass module (identical on all cores).

    fp8: x / G / Wv are fp8e4 and the four matmuls per chunk run in DoubleRow
    mode (contraction 256 in one pass).  Otherwise fp16.
    """
    if fp8 is None:
        fp8 = USE_FP8
    pairs = _pairs(ks)
    ncol = len(pairs)
    DT = F8 if fp8 else F16

    nc = bacc.Bacc("TRN2", target_bir_lowering=False, debug=False,
                   num_devices=N_CORES)
    if fp8:
        # DoubleRow operand layouts: [partition p, slot o, cols]; contraction
        # index i = o * 128 + p.  g/wv carry the fp8 "hi" part; gr/wvr the
        # fp8 residual (G_pre - hi), accumulated in a second DoubleRow pass.
        xt = nc.dram_tensor("xt", [B, 128, 2, ks], DT, kind="ExternalInput")
        g = nc.dram_tensor("g", [B, 128, 2, 256], DT, kind="ExternalInput")
        gr = nc.dram_tensor("gr", [B, 128, 2, 256], DT, kind="ExternalInput")
        wv = nc.dram_tensor("wv", [128, 2, 256], DT, kind="ExternalInput")
        wvr = nc.dram_tensor("wvr", [128, 2, 256], DT, kind="ExternalInput")
    else:
        # [b, i-half, i-partition, cols]
        xt = nc.dram_tensor("xt", [B, 2, 128, ks], DT, kind="ExternalInput")
        g = nc.dram_tensor("g", [B, 2, 128, 256], DT, kind="ExternalInput")
        wv = nc.dram_tensor("wv", [2, 128, 256], DT, kind="ExternalInput")
    oc = nc.dram_tensor("oc", [128, B, 2, 2, ncol], F32, kind="ExternalOutput")

    with ExitStack() as ctx:
        tc = ctx.enter_context(tile.TileContext(nc))
        wp = ctx.enter_context(tc.tile_pool(name="wp", bufs=1))
        xp = ctx.enter_context(tc.tile_pool(name="xp", bufs=1))
        pp = ctx.enter_context(tc.tile_pool(name="pp", bufs=2, space="PSUM"))
        ep = ctx.enter_context(tc.tile_pool(name="ep", bufs=3))
        sp = ctx.enter_context(tc.tile_pool(name="sp", bufs=2))
        ocp = ctx.enter_context(tc.tile_pool(name="ocp", bufs=2))

        if fp8:
            g_sb = wp.tile([128, 2, B, 2, 256], DT, tag="g")
            wv_sb = wp.tile([128, 2, 2, 256], DT, tag="wv")
            # one tile per batch so batch 0's matmuls only wait on its own DMA
            x_bt = [xp.tile([128, 2, ks], DT, tag=f"x{b}", name=f"x{b}")
                    for b in range(B)]
            # batch-0 weights first so they beat batch-0's x transfer
            nc.scalar.dma_start(out=g_sb[:, 0, 0], in_=g[0])
            nc.scalar.dma_start(out=g_sb[:, 1, 0], in_=gr[0])
            nc.scalar.dma_start(out=wv_sb[:, 0], in_=wv[:, :, :])
            nc.scalar.dma_start(out=wv_sb[:, 1], in_=wvr[:, :, :])
            for b in range(B):
                nc.sync.dma_start(out=x_bt[b][:, :, :], in_=xt[b])
                if b > 0:
                    nc.scalar.dma_start(out=g_sb[:, 0, b], in_=g[b])
                    nc.scalar.dma_start(out=g_sb[:, 1, b], in_=gr[b])

            def mm_s(out_ap, b, h, c0, csz):
                for r in range(2):
                    nc.tensor.matmul(
                        out_ap,
                        g_sb[:, r, b, :, h * 128:(h + 1) * 128],
                        x_bt[b][:, :, c0:c0 + csz],
                        start=(r == 0), stop=(r == 1),
                        perf_mode=mybir.MatmulPerfMode.DoubleRow)

            def mm_v(out_ap, b, h, c0, csz):
                for r in range(2):
                    nc.tensor.matmul(
                        out_ap,
                        wv_sb[:, r, :, h * 128:(h + 1) * 128],
                        x_bt[b][:, :, c0:c0 + csz],
                        start=(r == 0), stop=(r == 1),
                        perf_mode=mybir.MatmulPerfMode.DoubleRow)
        else:
            g_sb = wp.tile([128, B, 2, 256], DT, tag="g")
            wv_sb = wp.tile([128, 2, 256], DT, tag="wv")
            x_bt = [xp.tile([128, 2, ks], DT, tag=f"x{b}", name=f"x{b}")
                    for b in range(B)]
            for ih in range(2):
                nc.scalar.dma_start(out=g_sb[:, 0, ih], in_=g[0, ih])
            for ih in range(2):
                nc.scalar.dma_start(out=wv_sb[:, ih], in_=wv[ih])
            for b in range(B):
                for ih in range(2):
                    nc.sync.dma_start(out=x_bt[b][:, ih], in_=xt[b, ih])
                    if b > 0:
                        nc.scalar.dma_start(out=g_sb[:, b, ih], in_=g[b, ih])

            def mm_s(out_ap, b, h, c0, csz):
                for ih in range(2):
                    nc.tensor.matmul(
                        out_ap,
                        g_sb[:, b, ih, h * 128:(h + 1) * 128],
                        x_bt[b][:, ih, c0:c0 + csz],
                        start=(ih == 0), stop=(ih == 1))

            def mm_v(out_ap, b, h, c0, csz):
                for ih in range(2):
                    nc.tensor.matmul(
                        out_ap,
                        wv_sb[:, ih, h * 128:(h + 1) * 128],
                        x_bt[b][:, ih, c0:c0 + csz],
                        start=(ih == 0), stop=(ih == 1))

        for b in range(B):
            occ = ocp.tile([128, 2, 2, ncol], F32, tag="occ")
            # q-halves sequential so each PSUM tile spans a k-chunk PAIR
            # (2 banks): ScalarE/VectorE ops run 1024-wide, halving their
            # fixed per-op overhead.  4 tags x 2 banks = all 8 PSUM banks.
            for h in range(2):
                for t, sub in enumerate(pairs):
                    w = sum(cs for _, cs in sub)   # 1024 except the tail pair
                    s_p = pp.tile([128, 2 * CHUNK], F32, tag="s")
                    v_p = pp.tile([128, 2 * CHUNK], F32, tag="v")
                    for c, (c0, cs) in enumerate(sub):
                        mm_s(s_p[:, c * CHUNK:c * CHUNK + cs], b, h, c0, cs)
                        mm_v(v_p[:, c * CHUNK:c * CHUNK + cs], b, h, c0, cs)
                    e_p = ep.tile([128, 2 * CHUNK], F16, tag="e")
                    nc.scalar.activation(
                        e_p[:, :w], s_p[:, :w], AF.Exp,
                        scale=1.0 / PRESCALE,
                        accum_out=occ[:, h, 1, t:t + 1])
                    # numer: fused multiply+sum on VectorE via the standard
                    # TensorScalarPtr instruction (GpSimd cannot read PSUM on
                    # real HW; the custom tensor_tensor_reduce faults there)
                    p_ = sp.tile([128, 2 * CHUNK], F16, tag="p")
                    nc.vector.scalar_tensor_tensor(
                        out=p_[:, :w],
                        in0=v_p[:, :w], scalar=1.0, in1=e_p[:, :w],
                        op0=ALU.mult, op1=ALU.mult,
                        accum_out=occ[:, h, 0, t:t + 1])
            nc.sync.dma_start(out=oc[b], in_=occ[:, :, :, :])
    nc.compile()
    return nc


def _to_fp8(a):
    return np.clip(a, -240.0, 240.0).astype(ml_dtypes.float8_e4m3)


def _prepare_inputs(query, input, Wq, bq, Wk, Wv, fp8=False):
    """Host-side marshalling: G (incl. bq, 1/16, PRESCALE), Wv.T, x.T shards."""
    Q = query.astype(np.float64) @ Wq.T.astype(np.float64) + bq
    G = np.einsum('di,bqd->biq', Wk.astype(np.float64), Q) * (NORM * PRESCALE)
    WvT = Wv.T.astype(np.float64) * PRESCALE           # [i, j]

    xpad = np.zeros((B, LK_PAD, KV), np.float32)
    xpad[:, :LK] = input
    xT = xpad.transpose(0, 2, 1)                       # [B, 256, LK_PAD] view

    if fp8:
        def hires(a):  # [.., 2slots, ..] DoubleRow layout + residual split
            hi = _to_fp8(a)
            res = _to_fp8(a - hi.astype(np.float64))
            return np.ascontiguousarray(hi), np.ascontiguousarray(res)

        g8, gr8 = hires(G.reshape(B, 2, 128, 256).transpose(0, 2, 1, 3))
        wv8, wvr8 = hires(WvT.reshape(2, 128, 256).transpose(1, 0, 2))
        in_maps = []
        for c in range(N_CORES):
            sl = slice(c * KS, (c + 1) * KS)
            xc = xT[:, :, sl].reshape(B, 2, 128, KS).transpose(0, 2, 1, 3)
            in_maps.append({"xt": np.ascontiguousarray(_to_fp8(xc)),
                            "g": g8, "gr": gr8, "wv": wv8, "wvr": wvr8})
    else:
        g16 = np.ascontiguousarray(
            G.astype(np.float32).astype(np.float16).reshape(B, 2, 128, 256))
        wv16 = np.ascontiguousarray(
            WvT.astype(np.float32).astype(np.float16).reshape(2, 128, 256))
        in_maps = []
        for c in range(N_CORES):
            sl = slice(c * KS, (c + 1) * KS)
            xc = xT[:, :, sl].reshape(B, 2, 128, KS)
            in_maps.append({"xt": np.ascontiguousarray(xc.astype(np.float16)),
                            "g": g16, "wv": wv16})
    return in_maps


USE_FP8 = True


def kernel(query, input, Wq, bq, Wk, bk, Wv, bv):
    # bk provably cancels in softmax over k; bq is folded into G; bv is applied
    # in the host-side epilogue below.
    query = np.asarray(query, dtype=np.float32)
    input = np.asarray(input, dtype=np.float32)
    Wq = np.asarray(Wq, dtype=np.float32)
    bq = np.asarray(bq, dtype=np.float32)
    Wk = np.asarray(Wk, dtype=np.float32)
    Wv = np.asarray(Wv, dtype=np.float32)
    bv = np.asarray(bv, dtype=np.float32)

    nc = build(fp8=USE_FP8)
    in_maps = _prepare_inputs(query, input, Wq, bq, Wk, Wv, fp8=USE_FP8)
    res = run_bass_kernel_spmd(nc, in_maps, core_ids=list(range(N_CORES)))
    kernel._last_result = res

    numer = np.zeros((B, 2, 128))
    denom = np.zeros((B, 2, 128))
    for r in res.results:
        o = r["oc"].astype(np.float64)       # [B, 128, 2, 2, ncol]
        numer += o[:, :, :, 0, :].sum(axis=3).transpose(0, 2, 1)
        denom += o[:, :, :, 1, :].sum(axis=3).transpose(0, 2, 1)
    numer = numer.reshape(B, OUT) / PRESCALE
    denom = denom.reshape(B, OUT) - N_PAD    # padded rows contribute e=1 each
    out = numer / denom + bv
    return out.astype(np.float32)


if __name__ == "__main__":
    # CoreSim smoke test on a reduced size (2.25 chunks -> [512, 512, 128]).
    from concourse.bass_interp import CoreSim

    for fp8 in (False, True):
        ks = 1152
        rng = np.random.default_rng(0)
        x = rng.standard_normal((B, ks, KV)).astype(np.float32)
        G = (rng.standard_normal((B, KV, 256)) * 0.4).astype(np.float64)
        WvT = (rng.standard_normal((KV, 256)) * 0.8).astype(np.float64)

        nc = build(ks=ks, fp8=fp8)
        sim = CoreSim(nc)
        xT = x.transpose(0, 2, 1)  # [B, 256, ks]
        if fp8:
            sim.tensor("xt")[:] = _to_fp8(
                xT.reshape(B, 2, 128, ks).transpose(0, 2, 1, 3))
            gdr = G.reshape(B, 2, 128, 256).transpose(0, 2, 1, 3)
            wdr = WvT.reshape(2, 128, 256).transpose(1, 0, 2)
            g_hi = _to_fp8(gdr)
            g_re = _to_fp8(gdr - g_hi.astype(np.float64))
            w_hi = _to_fp8(wdr)
            w_re = _to_fp8(wdr - w_hi.astype(np.float64))
            sim.tensor("g")[:] = g_hi
            sim.tensor("gr")[:] = g_re
            sim.tensor("wv")[:] = w_hi
            sim.tensor("wvr")[:] = w_re
            xq = _to_fp8(xT).astype(np.float64)
            gq = (g_hi.astype(np.float64) + g_re.astype(np.float64)
                  ).transpose(0, 2, 1, 3).reshape(B, 256, 256)
            wq = (w_hi.astype(np.float64) + w_re.astype(np.float64)
                  ).transpose(1, 0, 2).reshape(256, 256)
        else:
            sim.tensor("xt")[:] = xT.reshape(B, 2, 128, ks).astype(np.float16)
            sim.tensor("g")[:] = G.astype(np.float16).reshape(B, 2, 128, 256)
            sim.tensor("wv")[:] = WvT.astype(np.float16).reshape(2, 128, 256)
            xq = xT.astype(np.float16).astype(np.float64)
            gq = G.astype(np.float16).astype(np.float64)
            wq = WvT.astype(np.float16).astype(np.float64)
        sim.simulate()
        got = np.array(sim.tensor("oc")).astype(np.float64)  # [B,128,2,2,ncol]
        gnum = got[:, :, :, 0, :].sum(axis=3).transpose(0, 2, 1).reshape(B, 256)
        gden = got[:, :, :, 1, :].sum(axis=3).transpose(0, 2, 1).reshape(B, 256)

        wnum = np.zeros((B, 256))
        wden = np.zeros((B, 256))
        for b in range(B):
            s = (gq[b].T @ xq[b]) / PRESCALE          # [256 q, ks]
            e = np.exp(s)
            v = wq.T @ xq[b]                          # [256 j, ks]
            e16 = e.astype(np.float16).astype(np.float64)
            wnum[b] = (e16 * v).sum(axis=1)
            wden[b] = e16.sum(axis=1)
        en = np.abs(gnum - wnum).max() / np.abs(wnum).max()
        ed = np.abs(gden - wden).max() / np.abs(wden).max()
        print(f"fp8={fp8}: CoreSim numer rel err {en:.3e}, denom rel err {ed:.3e}")
        assert en < 2e-2 and ed < 2e-2, (en, ed)
    print("OK")


# revision 39
# speedup vs baseline: 1.4196x; 1.0414x over previous
"""Trainium2 Bass kernel for nn_Attention_9122510537215 (gnn_message_passing).

Math (per batch b):
    Q = query @ Wq.T + bq                  [LQ=256, 256]
    K = input @ Wk.T + bk                  [LK, 256]
    V = input @ Wv.T + bv                  [LK, 256]
    alpha = softmax_k(Q @ K.T / 16)        [256, LK]
    out[j] = sum_k alpha[j, k] * V[k, j]   [256]

Restructure vs the two-layout baseline:
  * bk shifts every score column by a constant along k -> cancels in softmax_k.
  * G[b] = Wk.T @ (query_b @ Wq.T + bq).T / 16, so s[q, k] = (G.T @ x.T)[q, k].
  * vT[j, k] = (Wv @ x.T)[j, k] is computed ON DEVICE from the SAME moving
    operand as the scores (x.T), with Wv.T stationary.  Then
        numer[j] = sum_k e[j, k] * vT[j, k],   denom[j] = sum_k e[j, k]
    and out = numer / denom + bv (bv applied on host; scores are O(1) so the
    softmax runs unnormalized without max-subtraction).
  * Only ONE layout of the input is shipped (x.T, features-on-partitions):
    half the HBM traffic of the baseline.  G and Wv.T are the only PE
    stationaries (8 loads per 512-column chunk, fully hidden), the moving
    stream is x.T.
  * e = exp(s) runs on ScalarE with a fused free-axis accumulate (denom for
    free).  numer's multiply+reduce is split between VectorE
    (tensor_tensor_reduce) and GpSimd (scalar_tensor_tensor) so no single
    engine bottlenecks.  G / Wv are pre-scaled by 16 on the host (exp applies
    scale=1/16; numer is divided by 16 on the host).

Distribution: the LK (node) axis is zero-padded to 50176 = 8 * 6272 and
sharded across the 8 NeuronCores; each core returns per-chunk column sums
[B, 128, 2(half), 2(numer/denom), NCOL] fp32 and the host reduces in float64.
Padded rows have x = 0 -> s = 0 -> e = 1 exactly, contributing 0 to numer and
+176 (total, last core only) to denom: subtracted exactly on the host.
"""

import numpy as np
from contextlib import ExitStack

import ml_dtypes

import concourse.bass as bass
import concourse.mybir as mybir
import concourse.tile as tile
from concourse import bacc
from concourse.bass_utils import run_bass_kernel_spmd

# Problem constants (hardcoded; kernel.py must be self-contained).
B = 4
LQ = 256
LK = 50000
OUT = 256
KV = 256            # input feature dim
NORM = 1.0 / 16.0   # 1/sqrt(OUT)
PRESCALE = 128.0    # host multiplies G and Wv by this; undone on device/host
                    # (keeps the fp8 hi+residual split in e4m3's normal range)

N_CORES = 8
KS = 6272                  # nodes per core per batch (49 * 128)
LK_PAD = KS * N_CORES      # 50176
N_PAD = LK_PAD - LK        # 176 zero rows, all on the last core
CHUNK = 512                # moving columns per PSUM bank

F16 = mybir.dt.float16
F32 = mybir.dt.float32
F8 = mybir.dt.float8e4

ALU = mybir.AluOpType
AF = mybir.ActivationFunctionType


def _pairs(ks):
    """k-range split into pairs of CHUNK-wide subchunks (one PSUM bank each)."""
    out = []
    c0 = 0
    while c0 < ks:
        sub = []
        for _ in range(2):
            if c0 < ks:
                sub.append((c0, min(CHUNK, ks - c0)))
                c0 += CHUNK
        out.append(sub)
    return out


def build(ks=KS, fp8=None):
    """Emit the per-core SPMD Bass module (identical on all cores).

    fp8: x / G / Wv are fp8e4 and the four matmuls per chunk run in DoubleRow
    mode (contraction 256 in one pass).  Otherwise fp16.
    """
    if fp8 is None:
        fp8 = USE_FP8
    pairs = _pairs(ks)
    ncol = len(pairs)
    DT = F8 if fp8 else F16

    nc = bacc.Bacc("TRN2", target_bir_lowering=False, debug=False,
                   num_devices=N_CORES)
    if fp8:
        # DoubleRow operand layouts: [partition p, slot o, cols]; contraction
        # index i = o * 128 + p.  g/wv carry the fp8 "hi" part; gr/wvr the
        # fp8 residual (G_pre - hi), accumulated in a second DoubleRow pass.
        xt = nc.dram_tensor("xt", [B, 128, 2, ks], DT, kind="ExternalInput")
        g = nc.dram_tensor("g", [B, 128, 2, 256], DT, kind="ExternalInput")
        gr = nc.dram_tensor("gr", [B, 128, 2, 256], DT, kind="ExternalInput")
        wv = nc.dram_tensor("wv", [128, 2, 256], DT, kind="ExternalInput")
        wvr = nc.dram_tensor("wvr", [128, 2, 256], DT, kind="ExternalInput")
    else:
        # [b, i-half, i-partition, cols]
        xt = nc.dram_tensor("xt", [B, 2, 128, ks], DT, kind="ExternalInput")
        g = nc.dram_tensor("g", [B, 2, 128, 256], DT, kind="ExternalInput")
        wv = nc.dram_tensor("wv", [2, 128, 256], DT, kind="ExternalInput")
    oc = nc.dram_tensor("oc", [128, B, 2, 2, ncol], F32, kind="ExternalOutput")

    with ExitStack() as ctx:
        tc = ctx.enter_context(tile.TileContext(nc))
        wp = ctx.enter_context(tc.tile_pool(name="wp", bufs=1))
        xp = ctx.enter_context(tc.tile_pool(name="xp", bufs=1))
        pp = ctx.enter_context(tc.tile_pool(name="pp", bufs=2, space="PSUM"))
        ep = ctx.enter_context(tc.tile_pool(name="ep", bufs=3))
        sp = ctx.enter_context(tc.tile_pool(name="sp", bufs=2))
        ocp = ctx.enter_context(tc.tile_pool(name="ocp", bufs=2))

        if fp8:
            g_sb = wp.tile([128, 2, B, 2, 256], DT, tag="g")
            wv_sb = wp.tile([128, 2, 2, 256], DT, tag="wv")
            # one tile per batch so batch 0's matmuls only wait on its own DMA
            x_bt = [xp.tile([128, 2, ks], DT, tag=f"x{b}", name=f"x{b}")
                    for b in range(B)]
            # ONE queue, priority order: batch-0's first pairs, then the
            # small weight tensors, then the bulk.  A single HWDGE queue
            # keeps the transfer order exactly as issued.
            x0cut = min(6 * CHUNK, ks)
            nc.sync.dma_start(out=x_bt[0][:, :, :x0cut], in_=xt[0, :, :, :x0cut])
            nc.sync.dma_start(out=g_sb[:, 0, 0], in_=g[0])
            nc.sync.dma_start(out=g_sb[:, 1, 0], in_=gr[0])
            nc.sync.dma_start(out=wv_sb[:, 0], in_=wv[:, :, :])
            nc.sync.dma_start(out=wv_sb[:, 1], in_=wvr[:, :, :])
            if x0cut < ks:
                nc.sync.dma_start(out=x_bt[0][:, :, x0cut:],
                                  in_=xt[0, :, :, x0cut:])
            for b in range(1, B):
                nc.sync.dma_start(out=x_bt[b][:, :, :], in_=xt[b])
                nc.sync.dma_start(out=g_sb[:, 0, b], in_=g[b])
                nc.sync.dma_start(out=g_sb[:, 1, b], in_=gr[b])

            def mm_pair(s_p, v_p, b, h, sub):
                # stationary-major order: each of the 4 stationaries (G hi,
                # G res, Wv hi, Wv res) streams both subchunks back-to-back,
                # so the PE loads 4 stationaries per pair instead of 8.
                # Per-bank PSUM groups: start on the hi pass, stop on res.
                for dst, wt in ((s_p, g_sb[:, :, b]), (v_p, wv_sb)):
                    for r in range(2):
                        for c, (c0, cs) in enumerate(sub):
                            nc.tensor.matmul(
                                dst[:, c * CHUNK:c * CHUNK + cs],
                                wt[:, r, :, h * 128:(h + 1) * 128],
                                x_bt[b][:, :, c0:c0 + cs],
                                start=(r == 0), stop=(r == 1),
                                perf_mode=mybir.MatmulPerfMode.DoubleRow)
        else:
            g_sb = wp.tile([128, B, 2, 256], DT, tag="g")
            wv_sb = wp.tile([128, 2, 256], DT, tag="wv")
            x_bt = [xp.tile([128, 2, ks], DT, tag=f"x{b}", name=f"x{b}")
                    for b in range(B)]
            x0cut = min(6 * CHUNK, ks)
            for ih in range(2):
                nc.sync.dma_start(out=x_bt[0][:, ih, :x0cut],
                                  in_=xt[0, ih, :, :x0cut])
            for ih in range(2):
                nc.sync.dma_start(out=g_sb[:, 0, ih], in_=g[0, ih])
            for ih in range(2):
                nc.sync.dma_start(out=wv_sb[:, ih], in_=wv[ih])
            if x0cut < ks:
                for ih in range(2):
                    nc.sync.dma_start(out=x_bt[0][:, ih, x0cut:],
                                      in_=xt[0, ih, :, x0cut:])
            for b in range(1, B):
                for ih in range(2):
                    nc.sync.dma_start(out=x_bt[b][:, ih], in_=xt[b, ih])
                    nc.sync.dma_start(out=g_sb[:, b, ih], in_=g[b, ih])

            def mm_pair(s_p, v_p, b, h, sub):
                for dst, wt in ((s_p, g_sb[:, b]), (v_p, wv_sb)):
                    for ih in range(2):
                        for c, (c0, cs) in enumerate(sub):
                            nc.tensor.matmul(
                                dst[:, c * CHUNK:c * CHUNK + cs],
                                wt[:, ih, h * 128:(h + 1) * 128],
                                x_bt[b][:, ih, c0:c0 + cs],
                                start=(ih == 0), stop=(ih == 1))

        # Warm up ScalarE's Exp table during the initial DMA wait.
        warm = ep.tile([128, 16], F16, tag="warm")
        nc.vector.memset(warm[:, :], 0.0)
        nc.scalar.activation(warm[:, :], warm[:, :], AF.Exp)

        occ = ocp.tile([128, B, 2, 2, ncol], F32, tag="occ")
        for b in range(B):
            # q-halves sequential so each PSUM tile spans a k-chunk PAIR
            # (2 banks): ScalarE/VectorE ops run 1024-wide, halving their
            # fixed per-op overhead.  4 tags x 2 banks = all 8 PSUM banks.
            for h in range(2):
                for t, sub in enumerate(pairs):
                    w = sum(cs for _, cs in sub)   # 1024 except the tail pair
                    s_p = pp.tile([128, 2 * CHUNK], F32, tag="s")
                    v_p = pp.tile([128, 2 * CHUNK], F32, tag="v")
                    mm_pair(s_p, v_p, b, h, sub)
                    # exp + denominator in one ScalarE pass (the fused
                    # accum_out costs a 187ns accumulator read; GpSimd can
                    # neither read PSUM nor free-axis-reduce, so ACT keeps it)
                    e_p = ep.tile([128, 2 * CHUNK], F16, tag="e")
                    nc.scalar.activation(
                        e_p[:, :w], s_p[:, :w], AF.Exp,
                        scale=1.0 / PRESCALE,
                        accum_out=occ[:, b, h, 1, t:t + 1])
                    # numer: fused multiply+sum on VectorE via the standard
                    # TensorScalarPtr instruction (GpSimd cannot read PSUM on
                    # real HW; the custom tensor_tensor_reduce faults there)
                    p_ = sp.tile([128, 2 * CHUNK], F16, tag="p")
                    nc.vector.scalar_tensor_tensor(
                        out=p_[:, :w],
                        in0=v_p[:, :w], scalar=1.0, in1=e_p[:, :w],
                        op0=ALU.mult, op1=ALU.mult,
                        accum_out=occ[:, b, h, 0, t:t + 1])
        nc.sync.dma_start(out=oc[:, :, :, :, :], in_=occ[:, :, :, :, :])
    nc.compile()
    return nc


def _to_fp8(a):
    return np.clip(a, -240.0, 240.0).astype(ml_dtypes.float8_e4m3)


def _prepare_inputs(query, input, Wq, bq, Wk, Wv, fp8=False):
    """Host-side marshalling: G (incl. bq, 1/16, PRESCALE), Wv.T, x.T shards."""
    Q = query.astype(np.float64) @ Wq.T.astype(np.float64) + bq
    G = np.einsum('di,bqd->biq', Wk.astype(np.float64), Q) * (NORM * PRESCALE)
    WvT = Wv.T.astype(np.float64) * PRESCALE           # [i, j]

    xpad = np.zeros((B, LK_PAD, KV), np.float32)
    xpad[:, :LK] = input
    xT = xpad.transpose(0, 2, 1)                       # [B, 256, LK_PAD] view

    if fp8:
        def hires(a):  # [.., 2slots, ..] DoubleRow layout + residual split
            hi = _to_fp8(a)
            res = _to_fp8(a - hi.astype(np.float64))
            return np.ascontiguousarray(hi), np.ascontiguousarray(res)

        g8, gr8 = hires(G.reshape(B, 2, 128, 256).transpose(0, 2, 1, 3))
        wv8, wvr8 = hires(WvT.reshape(2, 128, 256).transpose(1, 0, 2))
        in_maps = []
        for c in range(N_CORES):
            sl = slice(c * KS, (c + 1) * KS)
            xc = xT[:, :, sl].reshape(B, 2, 128, KS).transpose(0, 2, 1, 3)
            in_maps.append({"xt": np.ascontiguousarray(_to_fp8(xc)),
                            "g": g8, "gr": gr8, "wv": wv8, "wvr": wvr8})
    else:
        g16 = np.ascontiguousarray(
            G.astype(np.float32).astype(np.float16).reshape(B, 2, 128, 256))
        wv16 = np.ascontiguousarray(
            WvT.astype(np.float32).astype(np.float16).reshape(2, 128, 256))
        in_maps = []
        for c in range(N_CORES):
            sl = slice(c * KS, (c + 1) * KS)
            xc = xT[:, :, sl].reshape(B, 2, 128, KS)
            in_maps.append({"xt": np.ascontiguousarray(xc.astype(np.float16)),
                            "g": g16, "wv": wv16})
    return in_maps


USE_FP8 = True


def kernel(query, input, Wq, bq, Wk, bk, Wv, bv):
    # bk provably cancels in softmax over k; bq is folded into G; bv is applied
    # in the host-side epilogue below.
    query = np.asarray(query, dtype=np.float32)
    input = np.asarray(input, dtype=np.float32)
    Wq = np.asarray(Wq, dtype=np.float32)
    bq = np.asarray(bq, dtype=np.float32)
    Wk = np.asarray(Wk, dtype=np.float32)
    Wv = np.asarray(Wv, dtype=np.float32)
    bv = np.asarray(bv, dtype=np.float32)

    nc = build(fp8=USE_FP8)
    in_maps = _prepare_inputs(query, input, Wq, bq, Wk, Wv, fp8=USE_FP8)
    res = run_bass_kernel_spmd(nc, in_maps, core_ids=list(range(N_CORES)))
    kernel._last_result = res

    numer = np.zeros((B, 2, 128))
    denom = np.zeros((B, 2, 128))
    for r in res.results:
        o = r["oc"].astype(np.float64)       # [128, B, 2, 2, ncol]
        numer += o[:, :, :, 0, :].sum(axis=3).transpose(1, 2, 0)
        denom += o[:, :, :, 1, :].sum(axis=3).transpose(1, 2, 0)
    numer = numer.reshape(B, OUT) / PRESCALE
    denom = denom.reshape(B, OUT) - N_PAD    # padded rows contribute e=1 each
    out = numer / denom + bv
    return out.astype(np.float32)


if __name__ == "__main__":
    # CoreSim smoke test on a reduced size (2.25 chunks -> [512, 512, 128]).
    from concourse.bass_interp import CoreSim

    for fp8 in (False, True):
        ks = 1152
        rng = np.random.default_rng(0)
        x = rng.standard_normal((B, ks, KV)).astype(np.float32)
        G = (rng.standard_normal((B, KV, 256)) * 0.4).astype(np.float64)
        WvT = (rng.standard_normal((KV, 256)) * 0.8).astype(np.float64)

        nc = build(ks=ks, fp8=fp8)
        sim = CoreSim(nc)
        xT = x.transpose(0, 2, 1)  # [B, 256, ks]
        if fp8:
            sim.tensor("xt")[:] = _to_fp8(
                xT.reshape(B, 2, 128, ks).transpose(0, 2, 1, 3))
            gdr = G.reshape(B, 2, 128, 256).transpose(0, 2, 1, 3)
            wdr = WvT.reshape(2, 128, 256).transpose(1, 0, 2)
            g_hi = _to_fp8(gdr)
            g_re = _to_fp8(gdr - g_hi.astype(np.float64))
            w_hi = _to_fp8(wdr)
            w_re = _to_fp8(wdr - w_hi.astype(np.float64))
            sim.tensor("g")[:] = g_hi
            sim.tensor("gr")[:] = g_re
            sim.tensor("wv")[:] = w_hi
            sim.tensor("wvr")[:] = w_re
            xq = _to_fp8(xT).astype(np.float64)
            gq = (g_hi.astype(np.float64) + g_re.astype(np.float64)
                  ).transpose(0, 2, 1, 3).reshape(B, 256, 256)
            wq = (w_hi.astype(np.float64) + w_re.astype(np.float64)
                  ).transpose(1, 0, 2).reshape(256, 256)
        else:
            sim.tensor("xt")[:] = xT.reshape(B, 2, 128, ks).astype(np.float16)
            sim.tensor("g")[:] = G.astype(np.float16).reshape(B, 2, 128, 256)
            sim.tensor("wv")[:] = WvT.astype(np.float16).reshape(2, 128, 256)
            xq = xT.astype(np.float16).astype(np.float64)
            gq = G.astype(np.float16).astype(np.float64)
            wq = WvT.astype(np.float16).astype(np.float64)
        sim.simulate()
        got = np.array(sim.tensor("oc")).astype(np.float64)  # [128,B,2,2,ncol]
        gnum = got[:, :, :, 0, :].sum(axis=3).transpose(1, 2, 0).reshape(B, 256)
        gden = got[:, :, :, 1, :].sum(axis=3).transpose(1, 2, 0).reshape(B, 256)

        wnum = np.zeros((B, 256))
        wden = np.zeros((B, 256))
        for b in range(B):
            s = (gq[b].T @ xq[b]) / PRESCALE          # [256 q, ks]
            e = np.exp(s)
            v = wq.T @ xq[b]                          # [256 j, ks]
            e16 = e.astype(np.float16).astype(np.float64)
            wnum[b] = (e16 * v).sum(axis=1)
            wden[b] = e16.sum(axis=1)
        en = np.abs(gnum - wnum).max() / np.abs(wnum).max()
        ed = np.abs(gden - wden).max() / np.abs(wden).max()
        print(f"fp8={fp8}: CoreSim numer rel err {en:.3e}, denom rel err {ed:.3e}")
        assert en < 2e-2 and ed < 2e-2, (en, ed)
    print("OK")


# revision 42
# speedup vs baseline: 1.4382x; 1.0131x over previous
"""Trainium2 Bass kernel for nn_Attention_9122510537215 (gnn_message_passing).

Math (per batch b):
    Q = query @ Wq.T + bq                  [LQ=256, 256]
    K = input @ Wk.T + bk                  [LK, 256]
    V = input @ Wv.T + bv                  [LK, 256]
    alpha = softmax_k(Q @ K.T / 16)        [256, LK]
    out[j] = sum_k alpha[j, k] * V[k, j]   [256]

Restructure vs the two-layout baseline:
  * bk shifts every score column by a constant along k -> cancels in softmax_k.
  * G[b] = Wk.T @ (query_b @ Wq.T + bq).T / 16, so s[q, k] = (G.T @ x.T)[q, k].
  * vT[j, k] = (Wv @ x.T)[j, k] is computed ON DEVICE from the SAME moving
    operand as the scores (x.T), with Wv.T stationary.  Then
        numer[j] = sum_k e[j, k] * vT[j, k],   denom[j] = sum_k e[j, k]
    and out = numer / denom + bv (bv applied on host; scores are O(1) so the
    softmax runs unnormalized without max-subtraction).
  * Only ONE layout of the input is shipped (x.T, features-on-partitions):
    half the HBM traffic of the baseline.  G and Wv.T are the only PE
    stationaries (8 loads per 512-column chunk, fully hidden), the moving
    stream is x.T.
  * e = exp(s) runs on ScalarE with a fused free-axis accumulate (denom for
    free).  numer's multiply+reduce is split between VectorE
    (tensor_tensor_reduce) and GpSimd (scalar_tensor_tensor) so no single
    engine bottlenecks.  G / Wv are pre-scaled by 16 on the host (exp applies
    scale=1/16; numer is divided by 16 on the host).

Distribution: the LK (node) axis is zero-padded to 50176 = 8 * 6272 and
sharded across the 8 NeuronCores; each core returns per-chunk column sums
[B, 128, 2(half), 2(numer/denom), NCOL] fp32 and the host reduces in float64.
Padded rows have x = 0 -> s = 0 -> e = 1 exactly, contributing 0 to numer and
+176 (total, last core only) to denom: subtracted exactly on the host.
"""

import numpy as np
from contextlib import ExitStack

import ml_dtypes

import concourse.bass as bass
import concourse.mybir as mybir
import concourse.tile as tile
from concourse import bacc
from concourse.bass_utils import run_bass_kernel_spmd

# Problem constants (hardcoded; kernel.py must be self-contained).
B = 4
LQ = 256
LK = 50000
OUT = 256
KV = 256            # input feature dim
NORM = 1.0 / 16.0   # 1/sqrt(OUT)
PRESCALE = 128.0    # host multiplies G and Wv by this; undone on device/host
                    # (keeps the fp8 hi+residual split in e4m3's normal range)

N_CORES = 8
KS = 6272                  # nodes per core per batch (49 * 128)
LK_PAD = KS * N_CORES      # 50176
N_PAD = LK_PAD - LK        # 176 zero rows, all on the last core
CHUNK = 512                # moving columns per PSUM bank

F16 = mybir.dt.float16
F32 = mybir.dt.float32
F8 = mybir.dt.float8e4

ALU = mybir.AluOpType
AF = mybir.ActivationFunctionType


def _pairs(ks):
    """k-range split into pairs of CHUNK-wide subchunks (one PSUM bank each)."""
    out = []
    c0 = 0
    while c0 < ks:
        sub = []
        for _ in range(2):
            if c0 < ks:
                sub.append((c0, min(CHUNK, ks - c0)))
                c0 += CHUNK
        out.append(sub)
    return out


def build(ks=KS, fp8=None):
    """Emit the per-core SPMD Bass module (identical on all cores).

    fp8: x / G / Wv are fp8e4 and the four matmuls per chunk run in DoubleRow
    mode (contraction 256 in one pass).  Otherwise fp16.
    """
    if fp8 is None:
        fp8 = USE_FP8
    pairs = _pairs(ks)
    # short (tail) pair first: it refills the cross-sweep pipeline ~4x
    # faster at each (batch, half) boundary; column order is irrelevant to
    # the host-side sum
    pairs.sort(key=lambda sub: sum(cs for _, cs in sub))
    ncol = len(pairs)
    DT = F8 if fp8 else F16

    nc = bacc.Bacc("TRN2", target_bir_lowering=False, debug=False,
                   num_devices=N_CORES)
    if fp8:
        # DoubleRow operand layouts: [partition p, slot o, cols]; contraction
        # index i = o * 128 + p.  g/wv carry the fp8 "hi" part; gr/wvr the
        # fp8 residual (G_pre - hi), accumulated in a second DoubleRow pass.
        xt = nc.dram_tensor("xt", [B, 128, 2, ks], DT, kind="ExternalInput")
        g = nc.dram_tensor("g", [B, 128, 2, 256], DT, kind="ExternalInput")
        gr = nc.dram_tensor("gr", [B, 128, 2, 256], DT, kind="ExternalInput")
        wv = nc.dram_tensor("wv", [128, 2, 256], DT, kind="ExternalInput")
        wvr = nc.dram_tensor("wvr", [128, 2, 256], DT, kind="ExternalInput")
    else:
        # [b, i-half, i-partition, cols]
        xt = nc.dram_tensor("xt", [B, 2, 128, ks], DT, kind="ExternalInput")
        g = nc.dram_tensor("g", [B, 2, 128, 256], DT, kind="ExternalInput")
        wv = nc.dram_tensor("wv", [2, 128, 256], DT, kind="ExternalInput")
    oc = nc.dram_tensor("oc", [128, B, 2, 2, ncol], F32, kind="ExternalOutput")

    with ExitStack() as ctx:
        tc = ctx.enter_context(tile.TileContext(nc))
        wp = ctx.enter_context(tc.tile_pool(name="wp", bufs=1))
        xp = ctx.enter_context(tc.tile_pool(name="xp", bufs=1))
        pp = ctx.enter_context(tc.tile_pool(name="pp", bufs=2, space="PSUM"))
        ep = ctx.enter_context(tc.tile_pool(name="ep", bufs=3))
        sp = ctx.enter_context(tc.tile_pool(name="sp", bufs=2))
        ocp = ctx.enter_context(tc.tile_pool(name="ocp", bufs=2))

        if fp8:
            g_sb = wp.tile([128, 2, B, 2, 256], DT, tag="g")
            wv_sb = wp.tile([128, 2, 2, 256], DT, tag="wv")
            # one tile per batch so batch 0's matmuls only wait on its own DMA
            x_bt = [xp.tile([128, 2, ks], DT, tag=f"x{b}", name=f"x{b}")
                    for b in range(B)]
            # ONE queue, priority order: batch-0's first pairs, then the
            # small weight tensors, then the bulk (in two pieces so pairs
            # land just ahead of compute).  A single HWDGE queue keeps the
            # transfer order exactly as issued.
            cuts = [c for c in (4 * CHUNK, 8 * CHUNK) if c < ks] + [ks]
            nc.sync.dma_start(out=x_bt[0][:, :, :cuts[0]],
                              in_=xt[0, :, :, :cuts[0]])
            nc.sync.dma_start(out=g_sb[:, 0, 0], in_=g[0])
            nc.sync.dma_start(out=g_sb[:, 1, 0], in_=gr[0])
            nc.sync.dma_start(out=wv_sb[:, 0], in_=wv[:, :, :])
            nc.sync.dma_start(out=wv_sb[:, 1], in_=wvr[:, :, :])
            for lo, hi in zip(cuts[:-1], cuts[1:]):
                nc.sync.dma_start(out=x_bt[0][:, :, lo:hi],
                                  in_=xt[0, :, :, lo:hi])
            for b in range(1, B):
                nc.sync.dma_start(out=x_bt[b][:, :, :], in_=xt[b])
                nc.sync.dma_start(out=g_sb[:, 0, b], in_=g[b])
                nc.sync.dma_start(out=g_sb[:, 1, b], in_=gr[b])

            def mm_pair(s_p, v_p, b, h, sub):
                # stationary-major order: each of the 4 stationaries (G hi,
                # G res, Wv hi, Wv res) streams both subchunks back-to-back,
                # so the PE loads 4 stationaries per pair instead of 8.
                # Per-bank PSUM groups: start on the hi pass, stop on res.
                for dst, wt in ((s_p, g_sb[:, :, b]), (v_p, wv_sb)):
                    for r in range(2):
                        for c, (c0, cs) in enumerate(sub):
                            nc.tensor.matmul(
                                dst[:, c * CHUNK:c * CHUNK + cs],
                                wt[:, r, :, h * 128:(h + 1) * 128],
                                x_bt[b][:, :, c0:c0 + cs],
                                start=(r == 0), stop=(r == 1),
                                perf_mode=mybir.MatmulPerfMode.DoubleRow)
        else:
            g_sb = wp.tile([128, B, 2, 256], DT, tag="g")
            wv_sb = wp.tile([128, 2, 256], DT, tag="wv")
            x_bt = [xp.tile([128, 2, ks], DT, tag=f"x{b}", name=f"x{b}")
                    for b in range(B)]
            cuts = [c for c in (4 * CHUNK, 8 * CHUNK) if c < ks] + [ks]
            for ih in range(2):
                nc.sync.dma_start(out=x_bt[0][:, ih, :cuts[0]],
                                  in_=xt[0, ih, :, :cuts[0]])
            for ih in range(2):
                nc.sync.dma_start(out=g_sb[:, 0, ih], in_=g[0, ih])
            for ih in range(2):
                nc.sync.dma_start(out=wv_sb[:, ih], in_=wv[ih])
            for lo, hi in zip(cuts[:-1], cuts[1:]):
                for ih in range(2):
                    nc.sync.dma_start(out=x_bt[0][:, ih, lo:hi],
                                      in_=xt[0, ih, :, lo:hi])
            for b in range(1, B):
                for ih in range(2):
                    nc.sync.dma_start(out=x_bt[b][:, ih], in_=xt[b, ih])
                    nc.sync.dma_start(out=g_sb[:, b, ih], in_=g[b, ih])

            def mm_pair(s_p, v_p, b, h, sub):
                for dst, wt in ((s_p, g_sb[:, b]), (v_p, wv_sb)):
                    for ih in range(2):
                        for c, (c0, cs) in enumerate(sub):
                            nc.tensor.matmul(
                                dst[:, c * CHUNK:c * CHUNK + cs],
                                wt[:, ih, h * 128:(h + 1) * 128],
                                x_bt[b][:, ih, c0:c0 + cs],
                                start=(ih == 0), stop=(ih == 1))

        # Warm up ScalarE's Exp table during the initial DMA wait.
        warm = ep.tile([128, 16], F16, tag="warm")
        nc.vector.memset(warm[:, :], 0.0)
        nc.scalar.activation(warm[:, :], warm[:, :], AF.Exp)

        occ = ocp.tile([128, B, 2, 2, ncol], F32, tag="occ")
        for b in range(B):
            # q-halves sequential so each PSUM tile spans a k-chunk PAIR
            # (2 banks): ScalarE/VectorE ops run 1024-wide, halving their
            # fixed per-op overhead.  4 tags x 2 banks = all 8 PSUM banks.
            for h in range(2):
                for t, sub in enumerate(pairs):
                    w = sum(cs for _, cs in sub)   # 1024 except the tail pair
                    s_p = pp.tile([128, 2 * CHUNK], F32, tag="s")
                    v_p = pp.tile([128, 2 * CHUNK], F32, tag="v")
                    mm_pair(s_p, v_p, b, h, sub)
                    # exp + denominator in one ScalarE pass (the fused
                    # accum_out costs a 187ns accumulator read; GpSimd can
                    # neither read PSUM nor free-axis-reduce, so ACT keeps it)
                    e_p = ep.tile([128, 2 * CHUNK], F16, tag="e")
                    nc.scalar.activation(
                        e_p[:, :w], s_p[:, :w], AF.Exp,
                        scale=1.0 / PRESCALE,
                        accum_out=occ[:, b, h, 1, t:t + 1])
                    # numer: fused multiply+sum on VectorE via the standard
                    # TensorScalarPtr instruction (GpSimd cannot read PSUM on
                    # real HW; the custom tensor_tensor_reduce faults there)
                    p_ = sp.tile([128, 2 * CHUNK], F16, tag="p")
                    nc.vector.scalar_tensor_tensor(
                        out=p_[:, :w],
                        in0=v_p[:, :w], scalar=1.0, in1=e_p[:, :w],
                        op0=ALU.mult, op1=ALU.mult,
                        accum_out=occ[:, b, h, 0, t:t + 1])
        nc.sync.dma_start(out=oc[:, :, :, :, :], in_=occ[:, :, :, :, :])
    nc.compile()
    return nc


def _to_fp8(a):
    return np.clip(a, -240.0, 240.0).astype(ml_dtypes.float8_e4m3)


def _prepare_inputs(query, input, Wq, bq, Wk, Wv, fp8=False):
    """Host-side marshalling: G (incl. bq, 1/16, PRESCALE), Wv.T, x.T shards."""
    Q = query.astype(np.float64) @ Wq.T.astype(np.float64) + bq
    G = np.einsum('di,bqd->biq', Wk.astype(np.float64), Q) * (NORM * PRESCALE)
    WvT = Wv.T.astype(np.float64) * PRESCALE           # [i, j]

    xpad = np.zeros((B, LK_PAD, KV), np.float32)
    xpad[:, :LK] = input
    xT = xpad.transpose(0, 2, 1)                       # [B, 256, LK_PAD] view

    if fp8:
        def hires(a):  # [.., 2slots, ..] DoubleRow layout + residual split
            hi = _to_fp8(a)
            res = _to_fp8(a - hi.astype(np.float64))
            return np.ascontiguousarray(hi), np.ascontiguousarray(res)

        g8, gr8 = hires(G.reshape(B, 2, 128, 256).transpose(0, 2, 1, 3))
        wv8, wvr8 = hires(WvT.reshape(2, 128, 256).transpose(1, 0, 2))
        in_maps = []
        for c in range(N_CORES):
            sl = slice(c * KS, (c + 1) * KS)
            xc = xT[:, :, sl].reshape(B, 2, 128, KS).transpose(0, 2, 1, 3)
            in_maps.append({"xt": np.ascontiguousarray(_to_fp8(xc)),
                            "g": g8, "gr": gr8, "wv": wv8, "wvr": wvr8})
    else:
        g16 = np.ascontiguousarray(
            G.astype(np.float32).astype(np.float16).reshape(B, 2, 128, 256))
        wv16 = np.ascontiguousarray(
            WvT.astype(np.float32).astype(np.float16).reshape(2, 128, 256))
        in_maps = []
        for c in range(N_CORES):
            sl = slice(c * KS, (c + 1) * KS)
            xc = xT[:, :, sl].reshape(B, 2, 128, KS)
            in_maps.append({"xt": np.ascontiguousarray(xc.astype(np.float16)),
                            "g": g16, "wv": wv16})
    return in_maps


USE_FP8 = True


def kernel(query, input, Wq, bq, Wk, bk, Wv, bv):
    # bk provably cancels in softmax over k; bq is folded into G; bv is applied
    # in the host-side epilogue below.
    query = np.asarray(query, dtype=np.float32)
    input = np.asarray(input, dtype=np.float32)
    Wq = np.asarray(Wq, dtype=np.float32)
    bq = np.asarray(bq, dtype=np.float32)
    Wk = np.asarray(Wk, dtype=np.float32)
    Wv = np.asarray(Wv, dtype=np.float32)
    bv = np.asarray(bv, dtype=np.float32)

    nc = build(fp8=USE_FP8)
    in_maps = _prepare_inputs(query, input, Wq, bq, Wk, Wv, fp8=USE_FP8)
    res = run_bass_kernel_spmd(nc, in_maps, core_ids=list(range(N_CORES)))
    kernel._last_result = res

    numer = np.zeros((B, 2, 128))
    denom = np.zeros((B, 2, 128))
    for r in res.results:
        o = r["oc"].astype(np.float64)       # [128, B, 2, 2, ncol]
        numer += o[:, :, :, 0, :].sum(axis=3).transpose(1, 2, 0)
        denom += o[:, :, :, 1, :].sum(axis=3).transpose(1, 2, 0)
    numer = numer.reshape(B, OUT) / PRESCALE
    denom = denom.reshape(B, OUT) - N_PAD    # padded rows contribute e=1 each
    out = numer / denom + bv
    return out.astype(np.float32)


if __name__ == "__main__":
    # CoreSim smoke test on a reduced size (2.25 chunks -> [512, 512, 128]).
    from concourse.bass_interp import CoreSim

    for fp8 in (False, True):
        ks = 1152
        rng = np.random.default_rng(0)
        x = rng.standard_normal((B, ks, KV)).astype(np.float32)
        G = (rng.standard_normal((B, KV, 256)) * 0.4).astype(np.float64)
        WvT = (rng.standard_normal((KV, 256)) * 0.8).astype(np.float64)

        nc = build(ks=ks, fp8=fp8)
        sim = CoreSim(nc)
        xT = x.transpose(0, 2, 1)  # [B, 256, ks]
        if fp8:
            sim.tensor("xt")[:] = _to_fp8(
                xT.reshape(B, 2, 128, ks).transpose(0, 2, 1, 3))
            gdr = G.reshape(B, 2, 128, 256).transpose(0, 2, 1, 3)
            wdr = WvT.reshape(2, 128, 256).transpose(1, 0, 2)
            g_hi = _to_fp8(gdr)
            g_re = _to_fp8(gdr - g_hi.astype(np.float64))
            w_hi = _to_fp8(wdr)
            w_re = _to_fp8(wdr - w_hi.astype(np.float64))
            sim.tensor("g")[:] = g_hi
            sim.tensor("gr")[:] = g_re
            sim.tensor("wv")[:] = w_hi
            sim.tensor("wvr")[:] = w_re
            xq = _to_fp8(xT).astype(np.float64)
            gq = (g_hi.astype(np.float64) + g_re.astype(np.float64)
                  ).transpose(0, 2, 1, 3).reshape(B, 256, 256)
            wq = (w_hi.astype(np.float64) + w_re.astype(np.float64)
                  ).transpose(1, 0, 2).reshape(256, 256)
        else:
            sim.tensor("xt")[:] = xT.reshape(B, 2, 128, ks).astype(np.float16)
            sim.tensor("g")[:] = G.astype(np.float16).reshape(B, 2, 128, 256)
            sim.tensor("wv")[:] = WvT.astype(np.float16).reshape(2, 128, 256)
            xq = xT.astype(np.float16).astype(np.float64)
            gq = G.astype(np.float16).astype(np.float64)
            wq = WvT.astype(np.float16).astype(np.float64)
        sim.simulate()
        got = np.array(sim.tensor("oc")).astype(np.float64)  # [128,B,2,2,ncol]
        gnum = got[:, :, :, 0, :].sum(axis=3).transpose(1, 2, 0).reshape(B, 256)
        gden = got[:, :, :, 1, :].sum(axis=3).transpose(1, 2, 0).reshape(B, 256)

        wnum = np.zeros((B, 256))
        wden = np.zeros((B, 256))
        for b in range(B):
            s = (gq[b].T @ xq[b]) / PRESCALE          # [256 q, ks]
            e = np.exp(s)
            v = wq.T @ xq[b]                          # [256 j, ks]
            e16 = e.astype(np.float16).astype(np.float64)
            wnum[b] = (e16 * v).sum(axis=1)
            wden[b] = e16.sum(axis=1)
        en = np.abs(gnum - wnum).max() / np.abs(wnum).max()
        ed = np.abs(gden - wden).max() / np.abs(wden).max()
        print(f"fp8={fp8}: CoreSim numer rel err {en:.3e}, denom rel err {ed:.3e}")
        assert en < 2e-2 and ed < 2e-2, (en, ed)
    print("OK")


# revision 51
# speedup vs baseline: 1.4775x; 1.0273x over previous
"""Trainium2 Bass kernel for nn_Attention_9122510537215 (gnn_message_passing).

Math (per batch b):
    Q = query @ Wq.T + bq                  [LQ=256, 256]
    K = input @ Wk.T + bk                  [LK, 256]
    V = input @ Wv.T + bv                  [LK, 256]
    alpha = softmax_k(Q @ K.T / 16)        [256, LK]
    out[j] = sum_k alpha[j, k] * V[k, j]   [256]

Restructure vs the two-layout baseline:
  * bk shifts every score column by a constant along k -> cancels in softmax_k.
  * G[b] = Wk.T @ (query_b @ Wq.T + bq).T / 16, so s[q, k] = (G.T @ x.T)[q, k].
  * vT[j, k] = (Wv @ x.T)[j, k] is computed ON DEVICE from the SAME moving
    operand as the scores (x.T), with Wv.T stationary.  Then
        numer[j] = sum_k e[j, k] * vT[j, k],   denom[j] = sum_k e[j, k]
    and out = numer / denom + bv (bv applied on host; scores are O(1) so the
    softmax runs unnormalized without max-subtraction).
  * Only ONE layout of the input is shipped (x.T, features-on-partitions) and
    only once, in fp8: a quarter of the baseline's HBM traffic.  G and Wv.T
    are the only PE stationaries; the moving stream is x.T in DoubleRow mode
    (contraction 256 per pass).
  * fp8 weight quantization error is killed with a hi+residual split: each
    scores/values matmul runs twice (fp8(W), then fp8(W - fp8(W))),
    accumulating in PSUM.  W is pre-scaled by 128 so the residuals stay in
    e4m3's normal range (exp applies scale=1/128; numer is /128 on host).
    End-to-end error ~2.8e-3, dominated by the fp8 x itself.
  * Loop: per (batch, q-half), k advances in uniform pairs of 448 columns
    (6272 = 14*448, no ragged tail), one PSUM bank per subchunk, all 8 banks
    double-buffered.  Per pair: 8 DoubleRow matmuls (TensorE), one 896-wide
    exp with fused denom accumulate (ScalarE), one 896-wide fused
    multiply+sum for numer (VectorE scalar_tensor_tensor).  Steady state is
    ScalarE-paced with zero gaps.
  * Known real-HW constraints honored: GpSimd cannot touch PSUM, the custom
    DVE tensor_tensor_reduce faults, matmuls are stationary-major ordered to
    minimize LDWEIGHTS pressure, and all DMAs sit on one HWDGE queue in
    priority order (batch-0 slices first, weights next, bulk last).

Distribution: the LK (node) axis is zero-padded to 50176 = 8 * 6272 and
sharded across the 8 NeuronCores; each core returns per-pair column sums
[128, B, 2(half), 2(numer/denom), 7] fp32 and the host reduces in float64.
Padded rows have x = 0 -> s = 0 -> e = 1 exactly, contributing 0 to numer and
+176 (total, last core only) to denom: subtracted exactly on the host.
"""

import numpy as np
from contextlib import ExitStack

import ml_dtypes

import concourse.mybir as mybir
import concourse.tile as tile
from concourse import bacc
from concourse.bass_utils import run_bass_kernel_spmd

# Problem constants (hardcoded; kernel.py must be self-contained).
B = 4
LQ = 256
LK = 50000
OUT = 256
KV = 256            # input feature dim
NORM = 1.0 / 16.0   # 1/sqrt(OUT)
PRESCALE = 128.0    # host multiplies G and Wv by this; undone on device/host
                    # (keeps the fp8 hi+residual split in e4m3's normal range)

N_CORES = 8
KS = 6272                  # nodes per core per batch (49 * 128)
LK_PAD = KS * N_CORES      # 50176
N_PAD = LK_PAD - LK        # 176 zero rows, all on the last core
CHUNK = 512                # PSUM bank width (fp32 columns)
SUB = 448                  # moving columns per matmul; 6272 = 14 * 448 makes
                           # every k-pair uniform (no ragged tail bubble)

F16 = mybir.dt.float16
F32 = mybir.dt.float32
F8 = mybir.dt.float8e4

ALU = mybir.AluOpType
AF = mybir.ActivationFunctionType


def _pairs(ks):
    """k-range split into pairs of SUB-wide subchunks (one PSUM bank each)."""
    out = []
    c0 = 0
    while c0 < ks:
        sub = []
        for _ in range(2):
            if c0 < ks:
                sub.append((c0, min(SUB, ks - c0)))
                c0 += SUB
        out.append(sub)
    for sub in out:
        # ops below run one [128, len(sub), cs] AP per pair: subchunk sizes
        # within a pair must match (only a trailing single-sub pair may be
        # short)
        assert len(sub) == 1 or sub[0][1] == sub[1][1], sub
    return out


def build(ks=KS, fp8=None):
    """Emit the per-core SPMD Bass module (identical on all cores).

    fp8: x / G / Wv are fp8e4 and the four matmuls per chunk run in DoubleRow
    mode (contraction 256 in one pass).  Otherwise fp16.
    """
    if fp8 is None:
        fp8 = USE_FP8
    pairs = _pairs(ks)
    ncol = len(pairs)
    DT = F8 if fp8 else F16

    nc = bacc.Bacc("TRN2", target_bir_lowering=False, debug=False,
                   num_devices=N_CORES)
    if fp8:
        # DoubleRow operand layouts: [partition p, slot o, cols]; contraction
        # index i = o * 128 + p.  g/wv carry the fp8 "hi" part; gr/wvr the
        # fp8 residual (G_pre - hi), accumulated in a second DoubleRow pass.
        xt = nc.dram_tensor("xt", [B, 128, 2, ks], DT, kind="ExternalInput")
        g = nc.dram_tensor("g", [B, 128, 2, 256], DT, kind="ExternalInput")
        gr = nc.dram_tensor("gr", [B, 128, 2, 256], DT, kind="ExternalInput")
        wv = nc.dram_tensor("wv", [128, 2, 256], DT, kind="ExternalInput")
        wvr = nc.dram_tensor("wvr", [128, 2, 256], DT, kind="ExternalInput")
    else:
        # [b, i-half, i-partition, cols]
        xt = nc.dram_tensor("xt", [B, 2, 128, ks], DT, kind="ExternalInput")
        g = nc.dram_tensor("g", [B, 2, 128, 256], DT, kind="ExternalInput")
        wv = nc.dram_tensor("wv", [2, 128, 256], DT, kind="ExternalInput")
    oc = nc.dram_tensor("oc", [128, B, 2, 2, ncol], F32, kind="ExternalOutput")

    with ExitStack() as ctx:
        tc = ctx.enter_context(tile.TileContext(nc))
        wp = ctx.enter_context(tc.tile_pool(name="wp", bufs=1))
        xp = ctx.enter_context(tc.tile_pool(name="xp", bufs=1))
        pp = ctx.enter_context(tc.tile_pool(name="pp", bufs=2, space="PSUM"))
        ep = ctx.enter_context(tc.tile_pool(name="ep", bufs=4))
        sp = ctx.enter_context(tc.tile_pool(name="sp", bufs=3))
        ocp = ctx.enter_context(tc.tile_pool(name="ocp", bufs=2))

        if fp8:
            g_sb = wp.tile([128, 2, B, 2, 256], DT, tag="g")
            wv_sb = wp.tile([128, 2, 2, 256], DT, tag="wv")
            # one tile per batch so batch 0's matmuls only wait on its own DMA
            x_bt = [xp.tile([128, 2, ks], DT, tag=f"x{b}", name=f"x{b}")
                    for b in range(B)]
            # ONE queue, priority order: batch-0's first pairs, then the
            # small weight tensors, then the bulk (in two pieces so pairs
            # land just ahead of compute).  A single HWDGE queue keeps the
            # transfer order exactly as issued.
            cuts = [c for c in (4 * CHUNK, 8 * CHUNK) if c < ks] + [ks]
            nc.sync.dma_start(out=x_bt[0][:, :, :cuts[0]],
                              in_=xt[0, :, :, :cuts[0]])
            nc.sync.dma_start(out=g_sb[:, 0, 0], in_=g[0])
            nc.sync.dma_start(out=g_sb[:, 1, 0], in_=gr[0])
            nc.sync.dma_start(out=wv_sb[:, 0], in_=wv[:, :, :])
            nc.sync.dma_start(out=wv_sb[:, 1], in_=wvr[:, :, :])
            for lo, hi in zip(cuts[:-1], cuts[1:]):
                nc.sync.dma_start(out=x_bt[0][:, :, lo:hi],
                                  in_=xt[0, :, :, lo:hi])
            for b in range(1, B):
                nc.sync.dma_start(out=x_bt[b][:, :, :], in_=xt[b])
                nc.sync.dma_start(out=g_sb[:, 0, b], in_=g[b])
                nc.sync.dma_start(out=g_sb[:, 1, b], in_=gr[b])

            def mm_pair(s_p, v_p, b, h, sub):
                # stationary-major order: each of the 4 stationaries (G hi,
                # G res, Wv hi, Wv res) streams both subchunks back-to-back,
                # so the PE loads 4 stationaries per pair instead of 8.
                # Per-bank PSUM groups: start on the hi pass, stop on res.
                for dst, wt in ((s_p, g_sb[:, :, b]), (v_p, wv_sb)):
                    for r in range(2):
                        for c, (c0, cs) in enumerate(sub):
                            nc.tensor.matmul(
                                dst[:, c, :cs],
                                wt[:, r, :, h * 128:(h + 1) * 128],
                                x_bt[b][:, :, c0:c0 + cs],
                                start=(r == 0), stop=(r == 1),
                                perf_mode=mybir.MatmulPerfMode.DoubleRow)
        else:
            g_sb = wp.tile([128, B, 2, 256], DT, tag="g")
            wv_sb = wp.tile([128, 2, 256], DT, tag="wv")
            x_bt = [xp.tile([128, 2, ks], DT, tag=f"x{b}", name=f"x{b}")
                    for b in range(B)]
            cuts = [c for c in (4 * CHUNK, 8 * CHUNK) if c < ks] + [ks]
            for ih in range(2):
                nc.sync.dma_start(out=x_bt[0][:, ih, :cuts[0]],
                                  in_=xt[0, ih, :, :cuts[0]])
            for ih in range(2):
                nc.sync.dma_start(out=g_sb[:, 0, ih], in_=g[0, ih])
            for ih in range(2):
                nc.sync.dma_start(out=wv_sb[:, ih], in_=wv[ih])
            for lo, hi in zip(cuts[:-1], cuts[1:]):
                for ih in range(2):
                    nc.sync.dma_start(out=x_bt[0][:, ih, lo:hi],
                                      in_=xt[0, ih, :, lo:hi])
            for b in range(1, B):
                for ih in range(2):
                    nc.sync.dma_start(out=x_bt[b][:, ih], in_=xt[b, ih])
                    nc.sync.dma_start(out=g_sb[:, b, ih], in_=g[b, ih])

            def mm_pair(s_p, v_p, b, h, sub):
                for dst, wt in ((s_p, g_sb[:, b]), (v_p, wv_sb)):
                    for ih in range(2):
                        for c, (c0, cs) in enumerate(sub):
                            nc.tensor.matmul(
                                dst[:, c, :cs],
                                wt[:, ih, h * 128:(h + 1) * 128],
                                x_bt[b][:, ih, c0:c0 + cs],
                                start=(ih == 0), stop=(ih == 1))

        # Warm up ScalarE's Exp table during the initial DMA wait.
        warm = ep.tile([128, 16], F16, tag="warm")
        nc.vector.memset(warm[:, :], 0.0)
        nc.scalar.activation(warm[:, :], warm[:, :], AF.Exp)

        occ = ocp.tile([128, B, 2, 2, ncol], F32, tag="occ")
        for b in range(B):
            # q-halves sequential so each PSUM tile spans a k-chunk PAIR
            # (2 banks): ScalarE/VectorE ops run 1024-wide, halving their
            # fixed per-op overhead.  4 tags x 2 banks = all 8 PSUM banks.
            for h in range(2):
                for t, sub in enumerate(pairs):
                    np_, cs = len(sub), sub[0][1]
                    s_p = pp.tile([128, 2, CHUNK], F32, tag="s")
                    v_p = pp.tile([128, 2, CHUNK], F32, tag="v")
                    mm_pair(s_p, v_p, b, h, sub)
                    # exp + denominator in one ScalarE pass (the fused
                    # accum_out costs a 187ns accumulator read; GpSimd can
                    # neither read PSUM nor free-axis-reduce, so ACT keeps it)
                    e_p = ep.tile([128, 2, CHUNK], F16, tag="e")
                    nc.scalar.activation(
                        e_p[:, :np_, :cs], s_p[:, :np_, :cs], AF.Exp,
                        scale=1.0 / PRESCALE,
                        accum_out=occ[:, b, h, 1, t:t + 1])
                    # numer: fused multiply+sum on VectorE via the standard
                    # TensorScalarPtr instruction (GpSimd cannot read PSUM on
                    # real HW; the custom tensor_tensor_reduce faults there)
                    p_ = sp.tile([128, 2, CHUNK], F16, tag="p")
                    nc.vector.scalar_tensor_tensor(
                        out=p_[:, :np_, :cs],
                        in0=v_p[:, :np_, :cs], scalar=1.0,
                        in1=e_p[:, :np_, :cs],
                        op0=ALU.mult, op1=ALU.mult,
                        accum_out=occ[:, b, h, 0, t:t + 1])
        nc.sync.dma_start(out=oc[:, :, :, :, :], in_=occ[:, :, :, :, :])
    nc.compile()
    return nc


def _to_fp8(a):
    return np.clip(a, -240.0, 240.0).astype(ml_dtypes.float8_e4m3)


def _prepare_inputs(query, input, Wq, bq, Wk, Wv, fp8=False):
    """Host-side marshalling: G (incl. bq, 1/16, PRESCALE), Wv.T, x.T shards."""
    Q = query.astype(np.float64) @ Wq.T.astype(np.float64) + bq
    G = np.einsum('di,bqd->biq', Wk.astype(np.float64), Q) * (NORM * PRESCALE)
    WvT = Wv.T.astype(np.float64) * PRESCALE           # [i, j]

    xpad = np.zeros((B, LK_PAD, KV), np.float32)
    xpad[:, :LK] = input
    xT = xpad.transpose(0, 2, 1)                       # [B, 256, LK_PAD] view

    if fp8:
        def hires(a):  # [.., 2slots, ..] DoubleRow layout + residual split
            hi = _to_fp8(a)
            res = _to_fp8(a - hi.astype(np.float64))
            return np.ascontiguousarray(hi), np.ascontiguousarray(res)

        g8, gr8 = hires(G.reshape(B, 2, 128, 256).transpose(0, 2, 1, 3))
        wv8, wvr8 = hires(WvT.reshape(2, 128, 256).transpose(1, 0, 2))
        in_maps = []
        for c in range(N_CORES):
            sl = slice(c * KS, (c + 1) * KS)
            xc = xT[:, :, sl].reshape(B, 2, 128, KS).transpose(0, 2, 1, 3)
            in_maps.append({"xt": np.ascontiguousarray(_to_fp8(xc)),
                            "g": g8, "gr": gr8, "wv": wv8, "wvr": wvr8})
    else:
        g16 = np.ascontiguousarray(
            G.astype(np.float32).astype(np.float16).reshape(B, 2, 128, 256))
        wv16 = np.ascontiguousarray(
            WvT.astype(np.float32).astype(np.float16).reshape(2, 128, 256))
        in_maps = []
        for c in range(N_CORES):
            sl = slice(c * KS, (c + 1) * KS)
            xc = xT[:, :, sl].reshape(B, 2, 128, KS)
            in_maps.append({"xt": np.ascontiguousarray(xc.astype(np.float16)),
                            "g": g16, "wv": wv16})
    return in_maps


USE_FP8 = True


def kernel(query, input, Wq, bq, Wk, bk, Wv, bv):
    # bk provably cancels in softmax over k; bq is folded into G; bv is applied
    # in the host-side epilogue below.
    query = np.asarray(query, dtype=np.float32)
    input = np.asarray(input, dtype=np.float32)
    Wq = np.asarray(Wq, dtype=np.float32)
    bq = np.asarray(bq, dtype=np.float32)
    Wk = np.asarray(Wk, dtype=np.float32)
    Wv = np.asarray(Wv, dtype=np.float32)
    bv = np.asarray(bv, dtype=np.float32)

    nc = build(fp8=USE_FP8)
    in_maps = _prepare_inputs(query, input, Wq, bq, Wk, Wv, fp8=USE_FP8)
    res = run_bass_kernel_spmd(nc, in_maps, core_ids=list(range(N_CORES)))
    kernel._last_result = res

    numer = np.zeros((B, 2, 128))
    denom = np.zeros((B, 2, 128))
    for r in res.results:
        o = r["oc"].astype(np.float64)       # [128, B, 2, 2, ncol]
        numer += o[:, :, :, 0, :].sum(axis=3).transpose(1, 2, 0)
        denom += o[:, :, :, 1, :].sum(axis=3).transpose(1, 2, 0)
    numer = numer.reshape(B, OUT) / PRESCALE
    denom = denom.reshape(B, OUT) - N_PAD    # padded rows contribute e=1 each
    out = numer / denom + bv
    return out.astype(np.float32)


if __name__ == "__main__":
    # CoreSim smoke test on a reduced size (2.25 chunks -> [512, 512, 128]).
    from concourse.bass_interp import CoreSim

    for fp8 in (False, True):
        ks = 1152
        rng = np.random.default_rng(0)
        x = rng.standard_normal((B, ks, KV)).astype(np.float32)
        G = (rng.standard_normal((B, KV, 256)) * 0.4).astype(np.float64)
        WvT = (rng.standard_normal((KV, 256)) * 0.8).astype(np.float64)

        nc = build(ks=ks, fp8=fp8)
        sim = CoreSim(nc)
        xT = x.transpose(0, 2, 1)  # [B, 256, ks]
        if fp8:
            sim.tensor("xt")[:] = _to_fp8(
                xT.reshape(B, 2, 128, ks).transpose(0, 2, 1, 3))
            gdr = G.reshape(B, 2, 128, 256).transpose(0, 2, 1, 3)
            wdr = WvT.reshape(2, 128, 256).transpose(1, 0, 2)
            g_hi = _to_fp8(gdr)
            g_re = _to_fp8(gdr - g_hi.astype(np.float64))
            w_hi = _to_fp8(wdr)
            w_re = _to_fp8(wdr - w_hi.astype(np.float64))
            sim.tensor("g")[:] = g_hi
            sim.tensor("gr")[:] = g_re
            sim.tensor("wv")[:] = w_hi
            sim.tensor("wvr")[:] = w_re
            xq = _to_fp8(xT).astype(np.float64)
            gq = (g_hi.astype(np.float64) + g_re.astype(np.float64)
                  ).transpose(0, 2, 1, 3).reshape(B, 256, 256)
            wq = (w_hi.astype(np.float64) + w_re.astype(np.float64)
                  ).transpose(1, 0, 2).reshape(256, 256)
        else:
            sim.tensor("xt")[:] = xT.reshape(B, 2, 128, ks).astype(np.float16)
            sim.tensor("g")[:] = G.astype(np.float16).reshape(B, 2, 128, 256)
            sim.tensor("wv")[:] = WvT.astype(np.float16).reshape(2, 128, 256)
            xq = xT.astype(np.float16).astype(np.float64)
            gq = G.astype(np.float16).astype(np.float64)
            wq = WvT.astype(np.float16).astype(np.float64)
        sim.simulate()
        got = np.array(sim.tensor("oc")).astype(np.float64)  # [128,B,2,2,ncol]
        gnum = got[:, :, :, 0, :].sum(axis=3).transpose(1, 2, 0).reshape(B, 256)
        gden = got[:, :, :, 1, :].sum(axis=3).transpose(1, 2, 0).reshape(B, 256)

        wnum = np.zeros((B, 256))
        wden = np.zeros((B, 256))
        for b in range(B):
            s = (gq[b].T @ xq[b]) / PRESCALE          # [256 q, ks]
            e = np.exp(s)
            v = wq.T @ xq[b]                          # [256 j, ks]
            e16 = e.astype(np.float16).astype(np.float64)
            wnum[b] = (e16 * v).sum(axis=1)
            wden[b] = e16.sum(axis=1)
        en = np.abs(gnum - wnum).max() / np.abs(wnum).max()
        ed = np.abs(gden - wden).max() / np.abs(wden).max()
        print(f"fp8={fp8}: CoreSim numer rel err {en:.3e}, denom rel err {ed:.3e}")
        assert en < 2e-2 and ed < 2e-2, (en, ed)
    print("OK")


# revision 58
# speedup vs baseline: 1.4842x; 1.0045x over previous
"""Trainium2 Bass kernel for nn_Attention_9122510537215 (gnn_message_passing).

Math (per batch b):
    Q = query @ Wq.T + bq                  [LQ=256, 256]
    K = input @ Wk.T + bk                  [LK, 256]
    V = input @ Wv.T + bv                  [LK, 256]
    alpha = softmax_k(Q @ K.T / 16)        [256, LK]
    out[j] = sum_k alpha[j, k] * V[k, j]   [256]

Restructure vs the two-layout baseline:
  * bk shifts every score column by a constant along k -> cancels in softmax_k.
  * G[b] = Wk.T @ (query_b @ Wq.T + bq).T / 16, so s[q, k] = (G.T @ x.T)[q, k].
  * vT[j, k] = (Wv @ x.T)[j, k] is computed ON DEVICE from the SAME moving
    operand as the scores (x.T), with Wv.T stationary.  Then
        numer[j] = sum_k e[j, k] * vT[j, k],   denom[j] = sum_k e[j, k]
    and out = numer / denom + bv (bv applied on host; scores are O(1) so the
    softmax runs unnormalized without max-subtraction).
  * Only ONE layout of the input is shipped (x.T, features-on-partitions) and
    only once, in fp8: a quarter of the baseline's HBM traffic.  G and Wv.T
    are the only PE stationaries; the moving stream is x.T in DoubleRow mode
    (contraction 256 per pass).
  * fp8 weight quantization error is killed with a hi+residual split: each
    scores/values matmul runs twice (fp8(W), then fp8(W - fp8(W))),
    accumulating in PSUM.  W is pre-scaled by 128 so the residuals stay in
    e4m3's normal range (exp applies scale=1/128; numer is /128 on host).
    End-to-end error ~2.8e-3, dominated by the fp8 x itself.
  * Loop: per (batch, q-half), k advances in uniform pairs of 448 columns
    (6272 = 14*448, no ragged tail), one PSUM bank per subchunk, all 8 banks
    double-buffered.  Per pair: 8 DoubleRow matmuls (TensorE), one 896-wide
    exp with fused denom accumulate (ScalarE), one 896-wide fused
    multiply+sum for numer (VectorE scalar_tensor_tensor).  Steady state is
    ScalarE-paced with zero gaps.
  * Known real-HW constraints honored: GpSimd cannot touch PSUM, the custom
    DVE tensor_tensor_reduce faults, matmuls are stationary-major ordered to
    minimize LDWEIGHTS pressure, and all DMAs sit on one HWDGE queue in
    priority order (batch-0 slices first, weights next, bulk last).

Distribution: the LK (node) axis is zero-padded to 50176 = 8 * 6272 and
sharded across the 8 NeuronCores; each core returns per-pair column sums
[128, B, 2(half), 2(numer/denom), 7] fp32 and the host reduces in float64.
Padded rows have x = 0 -> s = 0 -> e = 1 exactly, contributing 0 to numer and
+176 (total, last core only) to denom: subtracted exactly on the host.
"""

import numpy as np
from contextlib import ExitStack

import ml_dtypes

import concourse.mybir as mybir
import concourse.tile as tile
from concourse import bacc
from concourse.bass_utils import run_bass_kernel_spmd

# Problem constants (hardcoded; kernel.py must be self-contained).
B = 4
LQ = 256
LK = 50000
OUT = 256
KV = 256            # input feature dim
NORM = 1.0 / 16.0   # 1/sqrt(OUT)
PRESCALE = 128.0    # host multiplies G and Wv by this; undone on device/host
                    # (keeps the fp8 hi+residual split in e4m3's normal range)

N_CORES = 8
KS = 6272                  # nodes per core per batch (49 * 128)
LK_PAD = KS * N_CORES      # 50176
N_PAD = LK_PAD - LK        # 176 zero rows, all on the last core
CHUNK = 512                # PSUM bank width (fp32 columns)
SUB = 448                  # moving columns per matmul; 6272 = 14 * 448 makes
                           # every k-pair uniform (no ragged tail bubble)

F16 = mybir.dt.float16
F32 = mybir.dt.float32
F8 = mybir.dt.float8e4

ALU = mybir.AluOpType
AF = mybir.ActivationFunctionType


def _pairs(ks):
    """k-range split into pairs of SUB-wide subchunks (one PSUM bank each)."""
    out = []
    c0 = 0
    while c0 < ks:
        sub = []
        for _ in range(2):
            if c0 < ks:
                sub.append((c0, min(SUB, ks - c0)))
                c0 += SUB
        out.append(sub)
    for sub in out:
        # ops below run one [128, len(sub), cs] AP per pair: subchunk sizes
        # within a pair must match (only a trailing single-sub pair may be
        # short)
        assert len(sub) == 1 or sub[0][1] == sub[1][1], sub
    return out


def build(ks=KS, fp8=None):
    """Emit the per-core SPMD Bass module (identical on all cores).

    fp8: x / G / Wv are fp8e4 and the four matmuls per chunk run in DoubleRow
    mode (contraction 256 in one pass).  Otherwise fp16.
    """
    if fp8 is None:
        fp8 = USE_FP8
    pairs = _pairs(ks)
    ncol = len(pairs)
    DT = F8 if fp8 else F16

    nc = bacc.Bacc("TRN2", target_bir_lowering=False, debug=False,
                   num_devices=N_CORES)
    if fp8:
        # DoubleRow operand layouts: [partition p, slot o, cols]; contraction
        # index i = o * 128 + p.  g/wv carry the fp8 "hi" part; gr/wvr the
        # fp8 residual (G_pre - hi), accumulated in a second DoubleRow pass.
        xt = nc.dram_tensor("xt", [B, 128, 2, ks], DT, kind="ExternalInput")
        g = nc.dram_tensor("g", [B, 128, 2, 256], DT, kind="ExternalInput")
        gr = nc.dram_tensor("gr", [B, 128, 2, 256], DT, kind="ExternalInput")
        wv = nc.dram_tensor("wv", [128, 2, 256], DT, kind="ExternalInput")
        wvr = nc.dram_tensor("wvr", [128, 2, 256], DT, kind="ExternalInput")
    else:
        # [b, i-half, i-partition, cols]
        xt = nc.dram_tensor("xt", [B, 2, 128, ks], DT, kind="ExternalInput")
        g = nc.dram_tensor("g", [B, 2, 128, 256], DT, kind="ExternalInput")
        wv = nc.dram_tensor("wv", [2, 128, 256], DT, kind="ExternalInput")
    oc = nc.dram_tensor("oc", [128, B, 2, 2, ncol], F32, kind="ExternalOutput")

    with ExitStack() as ctx:
        tc = ctx.enter_context(tile.TileContext(nc))
        wp = ctx.enter_context(tc.tile_pool(name="wp", bufs=1))
        xp = ctx.enter_context(tc.tile_pool(name="xp", bufs=1))
        pp = ctx.enter_context(tc.tile_pool(name="pp", bufs=2, space="PSUM"))
        ep = ctx.enter_context(tc.tile_pool(name="ep", bufs=4))
        sp = ctx.enter_context(tc.tile_pool(name="sp", bufs=3))
        ocp = ctx.enter_context(tc.tile_pool(name="ocp", bufs=2))

        if fp8:
            g_sb = wp.tile([128, 2, B, 2, 256], DT, tag="g")
            wv_sb = wp.tile([128, 2, 2, 256], DT, tag="wv")
            # one tile per batch so batch 0's matmuls only wait on its own DMA
            x_bt = [xp.tile([128, 2, ks], DT, tag=f"x{b}", name=f"x{b}")
                    for b in range(B)]
            # ONE queue, priority order: batch-0's first pairs, then the
            # small weight tensors, then the bulk (in two pieces so pairs
            # land just ahead of compute).  A single HWDGE queue keeps the
            # transfer order exactly as issued.
            cuts = [c for c in (4 * CHUNK, 8 * CHUNK) if c < ks] + [ks]
            nc.sync.dma_start(out=x_bt[0][:, :, :cuts[0]],
                              in_=xt[0, :, :, :cuts[0]])
            nc.sync.dma_start(out=g_sb[:, 0, 0], in_=g[0])
            nc.sync.dma_start(out=g_sb[:, 1, 0], in_=gr[0])
            nc.sync.dma_start(out=wv_sb[:, 0], in_=wv[:, :, :])
            nc.sync.dma_start(out=wv_sb[:, 1], in_=wvr[:, :, :])
            for lo, hi in zip(cuts[:-1], cuts[1:]):
                nc.sync.dma_start(out=x_bt[0][:, :, lo:hi],
                                  in_=xt[0, :, :, lo:hi])
            for b in range(1, B):
                nc.sync.dma_start(out=x_bt[b][:, :, :], in_=xt[b])
                nc.sync.dma_start(out=g_sb[:, 0, b], in_=g[b])
                nc.sync.dma_start(out=g_sb[:, 1, b], in_=gr[b])

            def mm_group(tiles, b, h, grp):
                # stationary-major across a GROUP of pairs: each of the 4
                # stationaries (G hi, G res, Wv hi, Wv res) streams every
                # subchunk of every pair in the group back-to-back, so the PE
                # loads 4 stationaries per group (2 per pair) — minimal real-
                # HW LDWEIGHTS pressure.  Per-bank PSUM groups: start on the
                # hi pass, stop on res.
                for di, wt in ((0, g_sb[:, :, b]), (1, wv_sb)):
                    for r in range(2):
                        for t, sub in grp:
                            for c, (c0, cs) in enumerate(sub):
                                nc.tensor.matmul(
                                    tiles[t][di][:, c, :cs],
                                    wt[:, r, :, h * 128:(h + 1) * 128],
                                    x_bt[b][:, :, c0:c0 + cs],
                                    start=(r == 0), stop=(r == 1),
                                    perf_mode=mybir.MatmulPerfMode.DoubleRow)
        else:
            g_sb = wp.tile([128, B, 2, 256], DT, tag="g")
            wv_sb = wp.tile([128, 2, 256], DT, tag="wv")
            x_bt = [xp.tile([128, 2, ks], DT, tag=f"x{b}", name=f"x{b}")
                    for b in range(B)]
            cuts = [c for c in (4 * CHUNK, 8 * CHUNK) if c < ks] + [ks]
            for ih in range(2):
                nc.sync.dma_start(out=x_bt[0][:, ih, :cuts[0]],
                                  in_=xt[0, ih, :, :cuts[0]])
            for ih in range(2):
                nc.sync.dma_start(out=g_sb[:, 0, ih], in_=g[0, ih])
            for ih in range(2):
                nc.sync.dma_start(out=wv_sb[:, ih], in_=wv[ih])
            for lo, hi in zip(cuts[:-1], cuts[1:]):
                for ih in range(2):
                    nc.sync.dma_start(out=x_bt[0][:, ih, lo:hi],
                                      in_=xt[0, ih, :, lo:hi])
            for b in range(1, B):
                for ih in range(2):
                    nc.sync.dma_start(out=x_bt[b][:, ih], in_=xt[b, ih])
                    nc.sync.dma_start(out=g_sb[:, b, ih], in_=g[b, ih])

            def mm_group(tiles, b, h, grp):
                for di, wt in ((0, g_sb[:, b]), (1, wv_sb)):
                    for ih in range(2):
                        for t, sub in grp:
                            for c, (c0, cs) in enumerate(sub):
                                nc.tensor.matmul(
                                    tiles[t][di][:, c, :cs],
                                    wt[:, ih, h * 128:(h + 1) * 128],
                                    x_bt[b][:, ih, c0:c0 + cs],
                                    start=(ih == 0), stop=(ih == 1))

        # Warm up ScalarE's Exp table during the initial DMA wait.
        warm = ep.tile([128, 16], F16, tag="warm")
        nc.vector.memset(warm[:, :], 0.0)
        nc.scalar.activation(warm[:, :], warm[:, :], AF.Exp)

        occ = ocp.tile([128, B, 2, 2, ncol], F32, tag="occ")

        def emit_tail(b, h, t, sub, s_p, v_p):
            np_, cs = len(sub), sub[0][1]
            # exp + denominator in one ScalarE pass; the fused accum_out
            # costs a 187ns ACT accumulator read, so for one pair per sweep
            # the denom moves to a DVE tensor_scalar instead (all-SBUF fp16
            # -> 4x mode, and DVE has slack) to balance the two pace-setting
            # engines.
            d_ap = occ[:, b, h, 1, t:t + 1]
            dve_denom = (t == 3)
            e_p = ep.tile([128, 2, CHUNK], F16, tag="e")
            nc.scalar.activation(
                e_p[:, :np_, :cs], s_p[:, :np_, :cs], AF.Exp,
                scale=1.0 / PRESCALE,
                accum_out=None if dve_denom else d_ap)
            # numer: fused multiply+sum on VectorE via the standard
            # TensorScalarPtr instruction (GpSimd cannot read PSUM on real
            # HW; the custom tensor_tensor_reduce faults there)
            p_ = sp.tile([128, 2, CHUNK], F16, tag="p")
            nc.vector.scalar_tensor_tensor(
                out=p_[:, :np_, :cs],
                in0=v_p[:, :np_, :cs], scalar=1.0,
                in1=e_p[:, :np_, :cs],
                op0=ALU.mult, op1=ALU.mult,
                accum_out=occ[:, b, h, 0, t:t + 1])
            if dve_denom:
                pd = sp.tile([128, 2, CHUNK], F16, tag="pd")
                # NB: on real HW op1 acts as the accumulate/reduce op
                # (CoreSim just sums); op1=add + scalar2=0.0 is correct
                # under both semantics.
                nc.vector.tensor_scalar(
                    out=pd[:, :np_, :cs], in0=e_p[:, :np_, :cs],
                    scalar1=1.0, scalar2=0.0, op0=ALU.mult,
                    op1=ALU.add, accum_out=d_ap)

        for b in range(B):
            # q-halves sequential so each PSUM tile spans a k-chunk PAIR
            # (2 banks): ScalarE/VectorE ops run 896-wide, halving their
            # fixed per-op overhead.  2 tags x 2 banks x 2 bufs = all 8 PSUM
            # banks.  Matmuls go out in 2-pair groups, stationary-major, so
            # the PE loads each stationary once per group.
            for h in range(2):
                for t0 in range(0, len(pairs), 2):
                    grp = [(t, pairs[t])
                           for t in range(t0, min(t0 + 2, len(pairs)))]
                    tiles = {}
                    for t, sub in grp:
                        s_p = pp.tile([128, 2, CHUNK], F32, tag="s",
                                      name=f"s{t}")
                        v_p = pp.tile([128, 2, CHUNK], F32, tag="v",
                                      name=f"v{t}")
                        tiles[t] = (s_p, v_p)
                    mm_group(tiles, b, h, grp)
                    for t, sub in grp:
                        emit_tail(b, h, t, sub, *tiles[t])
        nc.sync.dma_start(out=oc[:, :, :, :, :], in_=occ[:, :, :, :, :])
    nc.compile()
    return nc


def _to_fp8(a):
    return np.clip(a, -240.0, 240.0).astype(ml_dtypes.float8_e4m3)


def _prepare_inputs(query, input, Wq, bq, Wk, Wv, fp8=False):
    """Host-side marshalling: G (incl. bq, 1/16, PRESCALE), Wv.T, x.T shards."""
    Q = query.astype(np.float64) @ Wq.T.astype(np.float64) + bq
    G = np.einsum('di,bqd->biq', Wk.astype(np.float64), Q) * (NORM * PRESCALE)
    WvT = Wv.T.astype(np.float64) * PRESCALE           # [i, j]

    xpad = np.zeros((B, LK_PAD, KV), np.float32)
    xpad[:, :LK] = input
    xT = xpad.transpose(0, 2, 1)                       # [B, 256, LK_PAD] view

    if fp8:
        def hires(a):  # [.., 2slots, ..] DoubleRow layout + residual split
            hi = _to_fp8(a)
            res = _to_fp8(a - hi.astype(np.float64))
            return np.ascontiguousarray(hi), np.ascontiguousarray(res)

        g8, gr8 = hires(G.reshape(B, 2, 128, 256).transpose(0, 2, 1, 3))
        wv8, wvr8 = hires(WvT.reshape(2, 128, 256).transpose(1, 0, 2))
        in_maps = []
        for c in range(N_CORES):
            sl = slice(c * KS, (c + 1) * KS)
            xc = xT[:, :, sl].reshape(B, 2, 128, KS).transpose(0, 2, 1, 3)
            in_maps.append({"xt": np.ascontiguousarray(_to_fp8(xc)),
                            "g": g8, "gr": gr8, "wv": wv8, "wvr": wvr8})
    else:
        g16 = np.ascontiguousarray(
            G.astype(np.float32).astype(np.float16).reshape(B, 2, 128, 256))
        wv16 = np.ascontiguousarray(
            WvT.astype(np.float32).astype(np.float16).reshape(2, 128, 256))
        in_maps = []
        for c in range(N_CORES):
            sl = slice(c * KS, (c + 1) * KS)
            xc = xT[:, :, sl].reshape(B, 2, 128, KS)
            in_maps.append({"xt": np.ascontiguousarray(xc.astype(np.float16)),
                            "g": g16, "wv": wv16})
    return in_maps


USE_FP8 = True


def kernel(query, input, Wq, bq, Wk, bk, Wv, bv):
    # bk provably cancels in softmax over k; bq is folded into G; bv is applied
    # in the host-side epilogue below.
    query = np.asarray(query, dtype=np.float32)
    input = np.asarray(input, dtype=np.float32)
    Wq = np.asarray(Wq, dtype=np.float32)
    bq = np.asarray(bq, dtype=np.float32)
    Wk = np.asarray(Wk, dtype=np.float32)
    Wv = np.asarray(Wv, dtype=np.float32)
    bv = np.asarray(bv, dtype=np.float32)

    nc = build(fp8=USE_FP8)
    in_maps = _prepare_inputs(query, input, Wq, bq, Wk, Wv, fp8=USE_FP8)
    res = run_bass_kernel_spmd(nc, in_maps, core_ids=list(range(N_CORES)))
    kernel._last_result = res

    numer = np.zeros((B, 2, 128))
    denom = np.zeros((B, 2, 128))
    for r in res.results:
        o = r["oc"].astype(np.float64)       # [128, B, 2, 2, ncol]
        numer += o[:, :, :, 0, :].sum(axis=3).transpose(1, 2, 0)
        denom += o[:, :, :, 1, :].sum(axis=3).transpose(1, 2, 0)
    numer = numer.reshape(B, OUT) / PRESCALE
    denom = denom.reshape(B, OUT) - N_PAD    # padded rows contribute e=1 each
    out = numer / denom + bv
    return out.astype(np.float32)


if __name__ == "__main__":
    # CoreSim smoke test on a reduced size (2.25 chunks -> [512, 512, 128]).
    from concourse.bass_interp import CoreSim

    for fp8 in (False, True):
        ks = 1152
        rng = np.random.default_rng(0)
        x = rng.standard_normal((B, ks, KV)).astype(np.float32)
        G = (rng.standard_normal((B, KV, 256)) * 0.4).astype(np.float64)
        WvT = (rng.standard_normal((KV, 256)) * 0.8).astype(np.float64)

        nc = build(ks=ks, fp8=fp8)
        sim = CoreSim(nc)
        xT = x.transpose(0, 2, 1)  # [B, 256, ks]
        if fp8:
            sim.tensor("xt")[:] = _to_fp8(
                xT.reshape(B, 2, 128, ks).transpose(0, 2, 1, 3))
            gdr = G.reshape(B, 2, 128, 256).transpose(0, 2, 1, 3)
            wdr = WvT.reshape(2, 128, 256).transpose(1, 0, 2)
            g_hi = _to_fp8(gdr)
            g_re = _to_fp8(gdr - g_hi.astype(np.float64))
            w_hi = _to_fp8(wdr)
            w_re = _to_fp8(wdr - w_hi.astype(np.float64))
            sim.tensor("g")[:] = g_hi
            sim.tensor("gr")[:] = g_re
            sim.tensor("wv")[:] = w_hi
            sim.tensor("wvr")[:] = w_re
            xq = _to_fp8(xT).astype(np.float64)
            gq = (g_hi.astype(np.float64) + g_re.astype(np.float64)
                  ).transpose(0, 2, 1, 3).reshape(B, 256, 256)
            wq = (w_hi.astype(np.float64) + w_re.astype(np.float64)
                  ).transpose(1, 0, 2).reshape(256, 256)
        else:
            sim.tensor("xt")[:] = xT.reshape(B, 2, 128, ks).astype(np.float16)
            sim.tensor("g")[:] = G.astype(np.float16).reshape(B, 2, 128, 256)
            sim.tensor("wv")[:] = WvT.astype(np.float16).reshape(2, 128, 256)
            xq = xT.astype(np.float16).astype(np.float64)
            gq = G.astype(np.float16).astype(np.float64)
            wq = WvT.astype(np.float16).astype(np.float64)
        sim.simulate()
        got = np.array(sim.tensor("oc")).astype(np.float64)  # [128,B,2,2,ncol]
        gnum = got[:, :, :, 0, :].sum(axis=3).transpose(1, 2, 0).reshape(B, 256)
        gden = got[:, :, :, 1, :].sum(axis=3).transpose(1, 2, 0).reshape(B, 256)

        wnum = np.zeros((B, 256))
        wden = np.zeros((B, 256))
        for b in range(B):
            s = (gq[b].T @ xq[b]) / PRESCALE          # [256 q, ks]
            e = np.exp(s)
            v = wq.T @ xq[b]                          # [256 j, ks]
            e16 = e.astype(np.float16).astype(np.float64)
            wnum[b] = (e16 * v).sum(axis=1)
            wden[b] = e16.sum(axis=1)
        en = np.abs(gnum - wnum).max() / np.abs(wnum).max()
        ed = np.abs(gden - wden).max() / np.abs(wden).max()
        print(f"fp8={fp8}: CoreSim numer rel err {en:.3e}, denom rel err {ed:.3e}")
        assert en < 2e-2 and ed < 2e-2, (en, ed)
    print("OK")
